# revision 6
# baseline (speedup 1.0000x reference)
import numpy as np

B, L = 4, 2048
DIM, A_DIM, H_DIM, DEPTH = 32, 128, 256, 8
RFF, C_DIM = 64, 256
H = 512
PAD = 64
W = PAD + 1024 + PAD
NCH = 3
CH = W // NCH
P = 128
MAGIC = float(12582912.0)

_PROG = None


def _tf32(x):
    x = np.ascontiguousarray(x, np.float32)
    u = x.view(np.uint32)
    r = ((u.astype(np.uint64) + 0x1000 + ((u >> 13) & 1)) & ~np.uint64(0x1FFF))
    return r.astype(np.uint32).view(np.float32)


def _wnorm(w):
    n = np.sqrt(np.sum(w * w, axis=tuple(range(1, w.ndim)), keepdims=True))
    return (w / (n + 1e-8)).astype(np.float32)


def _prep_weights(inp):
    f32 = lambda x: np.asarray(x, np.float32)
    out = {}
    ca = np.float32(0.5 * np.sqrt(66.0 / 32.0))
    cb = np.float32(0.5 * np.sqrt(66.0 / 1.0))
    Wh = _wnorm(f32(inp['W_h']))
    out['WxT'] = _tf32((Wh[:, :DIM] * ca).T)
    out['sbias'] = np.ascontiguousarray(Wh[:, DIM] * cb)[:, None]

    out['freq'] = np.ascontiguousarray(f32(inp['rff_freq']))[:, None]
    out['phase'] = np.ascontiguousarray(f32(inp['rff_phase']) + 0.25)[:, None]
    SQ2 = np.float32(np.sqrt(2.0))
    out['WeT'] = _tf32(_wnorm(f32(inp['W_e'])).T)
    out['WlT'] = _tf32((_wnorm(f32(inp['W_label'])) / SQ2).T)
    out['wurow'] = np.ascontiguousarray((_wnorm(f32(inp['W_u'])) * SQ2))

    ca2 = np.float32(0.5 * np.sqrt(3.0))
    cb2 = np.float32(0.5 * np.sqrt(6.0))
    IS = np.float32(1.0 / 0.596)
    hgT, pcT, gfT, gbT, outT, dw = [], [], [], [], [], []
    for i in range(DEPTH):
        hg = _wnorm(f32(inp['hg_W'][i]))
        hg_eff = np.concatenate([hg[:, :H_DIM] * (ca2 * IS), hg[:, H_DIM:] * cb2], 1)
        hgT.append(_tf32(hg_eff.T))
        pc = _wnorm(f32(inp['projc_W'][i])) * (f32(inp['projc_gain'][i]) * IS)
        pcT.append(_tf32(pc.T))
        gfT.append(_tf32((_wnorm(f32(inp['gruf_W'][i])) * IS).T))
        gbT.append(_tf32((_wnorm(f32(inp['grub_W'][i])) * IS).T))
        sc = np.float32(0.3 / (np.sqrt(0.58) * np.sqrt(2.0) * 0.596))
        outT.append(_tf32((_wnorm(f32(inp['out_W'][i])) * sc).T))
        dw.append(_wnorm(f32(inp['dw_W'][i]))[:, 0, :])
    out['hgT'] = np.ascontiguousarray(np.stack(hgT))
    out['pcT'] = np.ascontiguousarray(np.stack(pcT))
    out['gfT'] = np.ascontiguousarray(np.stack(gfT))
    out['gbT'] = np.ascontiguousarray(np.stack(gbT))
    out['outT'] = np.ascontiguousarray(np.stack(outT))
    out['dw'] = np.ascontiguousarray(np.stack(dw, axis=1).reshape(H, DEPTH * 3))
    out['WoT'] = _tf32((_wnorm(f32(inp['Wo'])) * f32(inp['out_gain'])).T)
    return out


def _core_inputs(inp, w, b, half):
    g0 = half * 1024 - PAD
    x_pad = np.zeros((DIM, W), np.float32)
    a_pad = np.zeros((A_DIM, W), np.float32)
    mask = np.zeros((1, W), np.float32)
    lo, hi = max(g0, 0), min(g0 + W, L)
    x_pad[:, lo - g0:hi - g0] = np.asarray(inp['x'][b], np.float32)[:, lo:hi]
    a_pad[:, lo - g0:hi - g0] = np.asarray(inp['a'][b], np.float32)[:, lo:hi]
    mask[:, lo - g0:hi - g0] = 1.0
    return {
        'x': _tf32(x_pad),
        'a': _tf32(a_pad),
        'mask': np.ascontiguousarray(np.broadcast_to(mask, (P, W))),
        'tcol': np.full((RFF, 1), np.float32(inp['t'][b]), np.float32),
        'label': _tf32(np.tile(np.asarray(inp['label'][b], np.float32)[:, None], (1, 2))),
        'freq': w['freq'], 'phase': w['phase'],
        'WxT': w['WxT'], 'sbias': w['sbias'],
        'WeT': w['WeT'], 'WlT': w['WlT'], 'WoT': w['WoT'],
        'frow': w['freq'].T.copy(), 'prow': w['phase'].T.copy(),
        'wurow': w['wurow'],
        'dw': w['dw'], 'hgT': w['hgT'], 'pcT': w['pcT'],
        'gfT': w['gfT'], 'gbT': w['gbT'], 'outT': w['outT'],
        'ones': np.ones((P, P), np.float32),
    }


def _build_program():
    import concourse.mybir as mybir
    import concourse.tile as tile
    from concourse import bacc

    F32 = mybir.dt.float32
    F32R = mybir.dt.float32r
    AF = mybir.ActivationFunctionType
    OP = mybir.AluOpType

    nc = bacc.Bacc(None)

    dp = nc.declare_dram_parameter
    x_in = dp("x", [DIM, W], F32R, isOutput=False)
    a_in = dp("a", [A_DIM, W], F32R, isOutput=False)
    mask_in = dp("mask", [P, W], F32, isOutput=False)
    tcol_in = dp("tcol", [RFF, 1], F32, isOutput=False)
    label_in = dp("label", [5, 2], F32R, isOutput=False)
    freq_in = dp("freq", [RFF, 1], F32, isOutput=False)
    phase_in = dp("phase", [RFF, 1], F32, isOutput=False)
    WxT_in = dp("WxT", [DIM, H_DIM], F32R, isOutput=False)
    sbias_in = dp("sbias", [H_DIM, 1], F32, isOutput=False)
    WeT_in = dp("WeT", [RFF, C_DIM], F32R, isOutput=False)
    WlT_in = dp("WlT", [5, C_DIM], F32R, isOutput=False)
    frow_in = dp("frow", [1, RFF], F32, isOutput=False)
    prow_in = dp("prow", [1, RFF], F32, isOutput=False)
    wurow_in = dp("wurow", [1, RFF], F32, isOutput=False)
    WoT_in = dp("WoT", [H_DIM, DIM], F32R, isOutput=False)
    dw_in = dp("dw", [H, DEPTH * 3], F32, isOutput=False)
    hgT_in = dp("hgT", [DEPTH, 384, 1024], F32R, isOutput=False)
    pcT_in = dp("pcT", [DEPTH, C_DIM, 1024], F32R, isOutput=False)
    gfT_in = dp("gfT", [DEPTH, H, 1024], F32R, isOutput=False)
    gbT_in = dp("gbT", [DEPTH, H, 1024], F32R, isOutput=False)
    outT_in = dp("outT", [DEPTH, H, H_DIM], F32R, isOutput=False)
    ones_in = dp("ones", [P, P], F32R, isOutput=False)
    o_out = dp("o", [DIM, 1024], F32, isOutput=True)
    u_out = dp("u", [1, 1], F32, isOutput=True)

    TWO_PI = float(2.0 * np.pi)
    ALPHA = float(0.7 / np.sqrt(0.58))

    with tile.TileContext(nc) as tc:
        with (
            tc.tile_pool(name="const", bufs=1) as cpool,
            tc.tile_pool(name="state", bufs=1) as spool,
            tc.tile_pool(name="wts", bufs=1) as wpool,
            tc.tile_pool(name="work", bufs=1) as wk,
            tc.tile_pool(name="mm", bufs=6, space="PSUM") as pmm,
            tc.tile_pool(name="vec", bufs=2, space="PSUM") as pvec,
        ):
            ones = cpool.tile([P, P], F32R, tag="ones", name="ones")
            nc.sync.dma_start(ones[:], ones_in[:])
            mask = cpool.tile([P, W], F32, tag="mask", name="mask")
            nc.sync.dma_start(mask[:], mask_in[:])
            eps_col = cpool.tile([P, 1], F32, tag="eps", name="eps_col")
            nc.vector.memset(eps_col[:], 1e-4)
            dw_t = []
            for k in range(4):
                t = cpool.tile([P, DEPTH * 3], F32, tag=f"dw{k}", name=f"dwt{k}")
                nc.sync.dma_start(t[:], dw_in[k * P:(k + 1) * P, :])
                dw_t.append(t)
            a_t = cpool.tile([P, W], F32R, tag="a", name="a_t")
            nc.sync.dma_start(a_t[:], a_in[:])
            sbias = []
            for m in range(2):
                t = cpool.tile([P, 1], F32, tag=f"sb{m}", name=f"sbias{m}")
                nc.sync.dma_start(t[:], sbias_in[m * P:(m + 1) * P, :])
                sbias.append(t)

            tcol = cpool.tile([RFF, 1], F32, tag="tcol", name="tcol")
            nc.sync.dma_start(tcol[:], tcol_in[:])
            freq = cpool.tile([RFF, 1], F32, tag="freq", name="freq")
            nc.sync.dma_start(freq[:], freq_in[:])
            phase = cpool.tile([RFF, 1], F32, tag="phase", name="phase")
            nc.sync.dma_start(phase[:], phase_in[:])
            label2 = cpool.tile([5, 2], F32R, tag="label", name="label2")
            nc.sync.dma_start(label2[:], label_in[:])
            WeT = wk.tile([RFF, C_DIM], F32R, tag="scr", bufs=4, name="WeT")
            nc.sync.dma_start(WeT[:], WeT_in[:])
            WlT = wk.tile([5, C_DIM], F32R, tag="scr", bufs=4, name="WlT")
            nc.sync.dma_start(WlT[:], WlT_in[:])
            frow = cpool.tile([1, RFF], F32, tag="frow", name="frow")
            nc.sync.dma_start(frow[:], frow_in[:])
            prow = cpool.tile([1, RFF], F32, tag="prow", name="prow")
            nc.sync.dma_start(prow[:], prow_in[:])
            wurow = cpool.tile([1, RFF], F32, tag="wurow", name="wurow")
            nc.sync.dma_start(wurow[:], wurow_in[:])

            xarg = wk.tile([RFF, 1], F32, tag="zsc", bufs=5, name="xarg")
            nc.vector.scalar_tensor_tensor(xarg[:], freq[:], tcol[:], phase[:],
                                           OP.mult, OP.add)
            m1 = wk.tile([RFF, 1], F32, tag="zsc", bufs=5, name="m1")
            nc.vector.tensor_scalar_add(m1[:], xarg[:], MAGIC)
            m2 = wk.tile([RFF, 1], F32, tag="zsc", bufs=5, name="m2")
            nc.vector.tensor_scalar_sub(m2[:], m1[:], MAGIC)
            red = wk.tile([RFF, 1], F32, tag="zsc", bufs=5, name="red")
            nc.vector.tensor_sub(red[:], xarg[:], m2[:])
            fsin = wk.tile([RFF, 2], F32R, tag="zsc", bufs=5, name="fsin")
            nc.scalar.activation(fsin[:, 0:1], red[:], AF.Sin, scale=TWO_PI)
            nc.scalar.activation(fsin[:, 1:2], red[:], AF.Sin, scale=TWO_PI)

            xargr = wk.tile([1, RFF], F32, tag="zsc", bufs=5, name="xargr")
            nc.vector.scalar_tensor_tensor(xargr[:], frow[:], tcol[:1, :], prow[:],
                                           OP.mult, OP.add)
            m1r = wk.tile([1, RFF], F32, tag="zsc", bufs=5, name="m1r")
            nc.vector.tensor_scalar_add(m1r[:], xargr[:], MAGIC)
            m2r = wk.tile([1, RFF], F32, tag="zsc", bufs=5, name="m2r")
            nc.vector.tensor_scalar_sub(m2r[:], m1r[:], MAGIC)
            redr = wk.tile([1, RFF], F32, tag="zsc", bufs=5, name="redr")
            nc.vector.tensor_sub(redr[:], xargr[:], m2r[:])
            sinr = wk.tile([1, RFF], F32, tag="zsc", bufs=5, name="sinr")
            nc.scalar.activation(sinr[:], redr[:], AF.Sin, scale=TWO_PI)
            usum = wk.tile([1, RFF], F32, tag="zsc", bufs=5, name="usum")
            nc.vector.tensor_mul(usum[:], wurow[:], sinr[:])
            usb = wk.tile([1, 1], F32, tag="zsc", bufs=5, name="usb")
            nc.vector.tensor_reduce(usb[:], usum[:], mybir.AxisListType.X,
                                    OP.add)
            nc.sync.dma_start(u_out[:], usb[:])

            c_t = []
            for m in range(2):
                cps = pvec.tile([P, 2], F32, tag="vps", name=f"cps{m}")
                nc.tensor.matmul(cps[:], WeT[:, m * P:(m + 1) * P], fsin[:],
                                 start=True, stop=False)
                nc.tensor.matmul(cps[:], WlT[:, m * P:(m + 1) * P], label2[:],
                                 start=False, stop=True)
                ct = cpool.tile([P, 2], F32R, tag=f"c{m}", name=f"c_t{m}")
                nc.scalar.activation(ct[:], cps[:], AF.Silu)
                c_t.append(ct)

            cm_all = cpool.tile([P, DEPTH * 8], F32, tag="cm", name="cm_all")
            for i in range(DEPTH):
                pc0 = wpool.tile([P, 1024], F32R, tag="wgf", bufs=4,
                                 name=f"pc0_{i}")
                nc.sync.dma_start(pc0[:], pcT_in[i, 0:P, :])
                pc1 = wpool.tile([P, 1024], F32R, tag="wgf", bufs=4,
                                 name=f"pc1_{i}")
                nc.sync.dma_start(pc1[:], pcT_in[i, P:C_DIM, :])
                for m in range(8):
                    cps = pvec.tile([P, 2], F32, tag="vps", name=f"cmps{i}_{m}")
                    nc.tensor.matmul(cps[:], pc0[:, m * P:(m + 1) * P], c_t[0][:],
                                     start=True, stop=False)
                    nc.tensor.matmul(cps[:], pc1[:, m * P:(m + 1) * P], c_t[1][:],
                                     start=False, stop=True)
                    nc.scalar.activation(cm_all[:, i * 8 + m:i * 8 + m + 1],
                                         cps[:, 0:1], AF.Copy, bias=1.0)

            xt = wk.tile([DIM, W], F32R, tag="scr", bufs=4, name="xt")
            nc.sync.dma_start(xt[:], x_in[:])
            WxT = wk.tile([DIM, H_DIM], F32R, tag="scr", bufs=4, name="WxT")
            nc.sync.dma_start(WxT[:], WxT_in[:])
            h_t = [spool.tile([P, W], F32, tag=f"h{m}", name=f"h_t{m}")
                   for m in range(2)]
            for n in range(NCH):
                sl = slice(n * CH, (n + 1) * CH)
                for m in range(2):
                    ps = pmm.tile([P, CH], F32, tag="ps", name=f"st_ps{n}_{m}")
                    nc.tensor.matmul(ps[:], WxT[:, m * P:(m + 1) * P], xt[:, sl],
                                     start=True, stop=True)
                    nc.vector.scalar_tensor_tensor(h_t[m][:, sl], mask[:, sl],
                                                   sbias[m][:], ps[:],
                                                   OP.mult, OP.add)

            for i in range(DEPTH):
                hg_w = []
                for kk in range(3):
                    t = wpool.tile([P, 1024], F32R, tag="whg", bufs=3,
                                   name=f"hgw{i}_{kk}")
                    nc.sync.dma_start(t[:], hgT_in[i, kk * P:(kk + 1) * P, :])
                    hg_w.append(t)
                gf_w = []
                gb_w = []
                for kk in range(4):
                    t = wpool.tile([P, 1024], F32R, tag="wgf", bufs=4,
                                   name=f"gfw{i}_{kk}")
                    nc.sync.dma_start(t[:], gfT_in[i, kk * P:(kk + 1) * P, :])
                    gf_w.append(t)
                for kk in range(4):
                    t = wpool.tile([P, 1024], F32R, tag="wgb", bufs=4,
                                   name=f"gbw{i}_{kk}")
                    nc.sync.dma_start(t[:], gbT_in[i, kk * P:(kk + 1) * P, :])
                    gb_w.append(t)
                out_w = []
                for kk in range(4):
                    t = wpool.tile([P, H_DIM], F32R, tag="wout", bufs=4,
                                   name=f"outw{i}_{kk}")
                    nc.sync.dma_start(t[:], outT_in[i, kk * P:(kk + 1) * P, :])
                    out_w.append(t)

                hsq = []
                for m in range(2):
                    t = wk.tile([P, W], F32R, tag="scr", bufs=4,
                                name=f"hsq{i}_{m}")
                    nc.scalar.activation(t[:], h_t[m][:], AF.Square)
                    hsq.append(t)
                tsd = wk.tile([P, W], F32, tag="scr", bufs=4, name=f"tsd{i}")
                for n in range(NCH):
                    sl = slice(n * CH, (n + 1) * CH)
                    ps = pmm.tile([P, CH], F32, tag="ps", name=f"pn_ps{i}_{n}")
                    nc.tensor.matmul(ps[:], ones[:], hsq[0][:, sl],
                                     start=True, stop=False)
                    nc.tensor.matmul(ps[:], ones[:], hsq[1][:, sl],
                                     start=False, stop=True)
                    nc.scalar.activation(tsd[:, sl], ps[:], AF.Sqrt,
                                         bias=eps_col[:], scale=1.0 / H_DIM)
                rinv = wk.tile([P, W], F32, tag="rinv", name=f"rinv{i}")
                nc.vector.reciprocal(rinv[:], tsd[:])
                hsil = []
                for m in range(2):
                    xn = wk.tile([P, W], F32, tag="scr", bufs=4, name=f"xn{i}_{m}")
                    nc.vector.tensor_mul(xn[:], h_t[m][:], rinv[:])
                    t = wk.tile([P, W], F32R, tag="sca", bufs=4,
                                name=f"hsil{i}_{m}")
                    nc.scalar.activation(t[:], xn[:], AF.Silu)
                    hsil.append(t)

                rhs3 = [hsil[0], hsil[1], a_t]
                halo = [wk.tile([P, W + 2], F32, tag="hab", bufs=4,
                                name=f"halo{i}_{k}") for k in range(4)]
                for t in halo:
                    nc.vector.memset(t[:, 0:1], 0.0)
                    nc.vector.memset(t[:, W + 1:W + 2], 0.0)
                gsil = [wk.tile([P, W], F32, tag="gsil", bufs=4,
                                name=f"gsil{i}_{k}") for k in range(4)]
                for n in range(NCH):
                    sl = slice(n * CH, (n + 1) * CH)
                    for m in range(8):
                        ps = pmm.tile([P, CH], F32, tag="ps",
                                      name=f"hg_ps{i}_{n}_{m}")
                        for kk in range(3):
                            nc.tensor.matmul(ps[:],
                                             hg_w[kk][:, m * P:(m + 1) * P],
                                             rhs3[kk][:, sl],
                                             start=(kk == 0), stop=(kk == 2))
                        if m < 4:
                            nc.scalar.activation(
                                halo[m][:, 1 + n * CH:1 + (n + 1) * CH], ps[:],
                                AF.Copy)
                        else:
                            nc.scalar.activation(
                                gsil[m - 4][:, sl], ps[:], AF.Silu,
                                scale=cm_all[:, i * 8 + m:i * 8 + m + 1])

                h1a = []
                for k in range(4):
                    c0 = dw_t[k][:, i * 3 + 0:i * 3 + 1]
                    c1 = dw_t[k][:, i * 3 + 1:i * 3 + 2]
                    c2 = dw_t[k][:, i * 3 + 2:i * 3 + 3]
                    m1_ = wk.tile([P, W], F32, tag="scr", bufs=4,
                                  name=f"dwm{i}_{k}")
                    nc.gpsimd.tensor_scalar_mul(m1_[:], halo[k][:, 1:W + 1], c1)
                    t1 = wk.tile([P, W], F32, tag="scr", bufs=4,
                                 name=f"dwt1_{i}_{k}")
                    nc.vector.scalar_tensor_tensor(t1[:], halo[k][:, 0:W], c0,
                                                   m1_[:], OP.mult, OP.add)
                    t2 = wk.tile([P, W], F32, tag="scr", bufs=4,
                                 name=f"dwt2_{i}_{k}")
                    nc.vector.scalar_tensor_tensor(t2[:], halo[k][:, 2:W + 2], c2,
                                                   t1[:], OP.mult, OP.add)
                    t3 = wk.tile([P, W], F32, tag="scr", bufs=4,
                                 name=f"dwt3_{i}_{k}")
                    nc.vector.tensor_mul(t3[:], t2[:], mask[:])
                    t4 = wk.tile([P, W], F32R, tag="h1a", bufs=4,
                                 name=f"h1a{i}_{k}")
                    nc.scalar.activation(t4[:], t3[:], AF.Silu,
                                         scale=cm_all[:, i * 8 + k:i * 8 + k + 1])
                    h1a.append(t4)

                def gru_pass(g_w, reverse, li):
                    bco, aco = [], []
                    for k in range(4):
                        bco.append(wk.tile([P, W], F32, tag="bco", bufs=4,
                                           name=f"bco{li}_{reverse}_{k}"))
                        aco.append(wk.tile([P, W], F32, tag="sca", bufs=4,
                                           name=f"aco{li}_{reverse}_{k}"))
                    for n in range(NCH):
                        sl = slice(n * CH, (n + 1) * CH)
                        zs_ch = [None] * 4
                        for m in range(8):
                            ps = pmm.tile([P, CH], F32, tag="ps",
                                          name=f"g_ps{li}_{reverse}_{n}_{m}")
                            for kk in range(4):
                                nc.tensor.matmul(ps[:],
                                                 g_w[kk][:, m * P:(m + 1) * P],
                                                 h1a[kk][:, sl],
                                                 start=(kk == 0), stop=(kk == 3))
                            if m < 4:
                                zc = wk.tile([P, CH], F32, tag="zsc", bufs=5,
                                             name=f"zc{li}_{reverse}_{n}_{m}")
                                nc.scalar.activation(zc[:], ps[:], AF.Sigmoid)
                                zs_ch[m] = zc
                            else:
                                k = m - 4
                                nc.vector.tensor_mul(bco[k][:, sl],
                                                     zs_ch[k][:], ps[:])
                                nc.gpsimd.tensor_scalar(aco[k][:, sl],
                                                        zs_ch[k][:], -1.0, 1.0,
                                                        OP.mult, OP.add)
                    outs = []
                    for k in range(4):
                        o_ = wk.tile([P, W], F32,
                                     tag=("hab" if reverse else "fwd"), bufs=4,
                                     name=f"scan{li}_{reverse}_{k}")
                        if reverse:
                            nc.vector.tensor_tensor_scan(
                                o_[:, ::-1], aco[k][:, ::-1], bco[k][:, ::-1],
                                0.0, OP.mult, OP.add)
                        else:
                            nc.vector.tensor_tensor_scan(
                                o_[:], aco[k][:], bco[k][:], 0.0,
                                OP.mult, OP.add)
                        outs.append(o_)
                    return outs

                fwd = gru_pass(gf_w, False, i)
                bwd = gru_pass(gb_w, True, i)

                prod = []
                for k in range(4):
                    h1s = wk.tile([P, W], F32, tag="sca", bufs=4,
                                  name=f"h1s{i}_{k}")
                    nc.vector.tensor_add(h1s[:], fwd[k][:], bwd[k][:])
                    pr = wk.tile([P, W], F32R, tag="bco", bufs=4,
                                 name=f"prod{i}_{k}")
                    nc.vector.tensor_mul(pr[:], h1s[:], gsil[k][:])
                    prod.append(pr)
                xn2 = []
                for m in range(2):
                    t = wk.tile([P, W], F32, tag="scr", bufs=4, name=f"xn2_{i}_{m}")
                    nc.vector.tensor_mul(t[:], h_t[m][:], rinv[:])
                    xn2.append(t)
                for n in range(NCH):
                    sl = slice(n * CH, (n + 1) * CH)
                    for m in range(2):
                        ps = pmm.tile([P, CH], F32, tag="ps",
                                      name=f"o_ps{i}_{n}_{m}")
                        for kk in range(4):
                            nc.tensor.matmul(ps[:],
                                             out_w[kk][:, m * P:(m + 1) * P],
                                             prod[kk][:, sl],
                                             start=(kk == 0), stop=(kk == 3))
                        nc.vector.scalar_tensor_tensor(
                            h_t[m][:, sl], xn2[m][:, sl], ALPHA, ps[:],
                            OP.mult, OP.add)

            Wo_t = []
            for m in range(2):
                t = cpool.tile([P, DIM], F32R, tag=f"Wo{m}", name=f"Wo_t{m}")
                nc.sync.dma_start(t[:], WoT_in[m * P:(m + 1) * P, :])
                Wo_t.append(t)
            h_r = []
            for m in range(2):
                t = wk.tile([P, W], F32R, tag="scr", bufs=4, name=f"h_r{m}")
                nc.scalar.activation(t[:], h_t[m][:], AF.Copy)
                h_r.append(t)
            o_sb = wk.tile([DIM, 1024], F32, tag="scr", bufs=4, name="o_sb")
            for (c0, cw) in ((0, 384), (384, 384), (768, 256)):
                ps = pmm.tile([DIM, 384], F32, tag="ps", name=f"head_ps{c0}")
                for m in range(2):
                    nc.tensor.matmul(ps[:, :cw], Wo_t[m][:],
                                     h_r[m][:, PAD + c0:PAD + c0 + cw],
                                     start=(m == 0), stop=(m == 1))
                nc.scalar.activation(o_sb[:, c0:c0 + cw], ps[:, :cw], AF.Copy)
            nc.sync.dma_start(o_out[:], o_sb[:])

    nc.compile()
    return nc


def _get_prog():
    global _PROG
    if _PROG is None:
        _PROG = _build_program()
    return _PROG


def kernel(**inputs):
    from concourse.bass_utils import run_bass_kernel_spmd

    w = _prep_weights(inputs)
    in_maps = []
    for b in range(B):
        for half in (0, 1):
            in_maps.append(_core_inputs(inputs, w, b, half))
    nc = _get_prog()
    res = run_bass_kernel_spmd(nc, in_maps, list(range(8)))
    o = np.zeros((B, DIM, L), np.float32)
    u = np.zeros((B,), np.float32)
    for ci, r in enumerate(res.results):
        b, half = divmod(ci, 2)
        o[b, :, half * 1024:(half + 1) * 1024] = r["o"]
        if half == 0:
            u[b] = r["u"][0, 0]
    return o, u


# revision 8
# speedup vs baseline: 1.5255x; 1.5255x over previous
import numpy as np

B, L = 4, 2048
DIM, A_DIM, H_DIM, DEPTH = 32, 128, 256, 8
RFF, C_DIM = 64, 256
H = 512
PAD = 64
W = PAD + 1024 + PAD
NCH = 3
CH = W // NCH
P = 128
MAGIC = float(12582912.0)

_PROG = None


def _tf32(x):
    x = np.ascontiguousarray(x, np.float32)
    u = x.view(np.uint32)
    r = ((u.astype(np.uint64) + 0x1000 + ((u >> 13) & 1)) & ~np.uint64(0x1FFF))
    return r.astype(np.uint32).view(np.float32)


def _wnorm(w):
    n = np.sqrt(np.sum(w * w, axis=tuple(range(1, w.ndim)), keepdims=True))
    return (w / (n + 1e-8)).astype(np.float32)


def _prep_weights(inp):
    f32 = lambda x: np.asarray(x, np.float32)
    out = {}
    ca = np.float32(0.5 * np.sqrt(66.0 / 32.0))
    cb = np.float32(0.5 * np.sqrt(66.0 / 1.0))
    Wh = _wnorm(f32(inp['W_h']))
    out['WxT'] = _tf32((Wh[:, :DIM] * ca).T)
    out['sbias'] = np.ascontiguousarray(Wh[:, DIM] * cb)[:, None]

    out['freq'] = np.ascontiguousarray(f32(inp['rff_freq']))[:, None]
    out['phase'] = np.ascontiguousarray(f32(inp['rff_phase']) + 0.25)[:, None]
    SQ2 = np.float32(np.sqrt(2.0))
    out['WeT'] = _tf32(_wnorm(f32(inp['W_e'])).T)
    out['WlT'] = _tf32((_wnorm(f32(inp['W_label'])) / SQ2).T)
    out['wurow'] = np.ascontiguousarray((_wnorm(f32(inp['W_u'])) * SQ2))

    ca2 = np.float32(0.5 * np.sqrt(3.0))
    cb2 = np.float32(0.5 * np.sqrt(6.0))
    IS = np.float32(1.0 / 0.596)
    hgT, pcT, gfT, gbT, outT, dw = [], [], [], [], [], []
    for i in range(DEPTH):
        hg = _wnorm(f32(inp['hg_W'][i]))
        hg_eff = np.concatenate([hg[:, :H_DIM] * (ca2 * IS), hg[:, H_DIM:] * cb2], 1)
        hgT.append(_tf32(hg_eff.T))
        pc = _wnorm(f32(inp['projc_W'][i])) * (f32(inp['projc_gain'][i]) * IS)
        pcT.append(_tf32(pc.T))
        gfT.append(_tf32((_wnorm(f32(inp['gruf_W'][i])) * IS).T))
        gbT.append(_tf32((_wnorm(f32(inp['grub_W'][i])) * IS).T))
        sc = np.float32(0.3 / (np.sqrt(0.58) * np.sqrt(2.0) * 0.596))
        outT.append(_tf32((_wnorm(f32(inp['out_W'][i])) * sc).T))
        dw.append(_wnorm(f32(inp['dw_W'][i]))[:, 0, :])
    out['hgT'] = np.ascontiguousarray(np.stack(hgT))
    out['pcT'] = np.ascontiguousarray(np.stack(pcT))
    out['gfT'] = np.ascontiguousarray(np.stack(gfT))
    out['gbT'] = np.ascontiguousarray(np.stack(gbT))
    out['outT'] = np.ascontiguousarray(np.stack(outT))
    out['dw'] = np.ascontiguousarray(np.stack(dw, axis=1).reshape(H, DEPTH * 3))
    out['WoT'] = _tf32((_wnorm(f32(inp['Wo'])) * f32(inp['out_gain'])).T)
    return out


def _core_inputs(inp, w, b, half):
    g0 = half * 1024 - PAD
    x_pad = np.zeros((DIM, W), np.float32)
    a_pad = np.zeros((A_DIM, W), np.float32)
    mask = np.zeros((1, W), np.float32)
    lo, hi = max(g0, 0), min(g0 + W, L)
    x_pad[:, lo - g0:hi - g0] = np.asarray(inp['x'][b], np.float32)[:, lo:hi]
    a_pad[:, lo - g0:hi - g0] = np.asarray(inp['a'][b], np.float32)[:, lo:hi]
    mask[:, lo - g0:hi - g0] = 1.0
    return {
        'x': _tf32(x_pad),
        'a': _tf32(a_pad),
        'mask': np.ascontiguousarray(np.broadcast_to(mask, (P, W))),
        'tcol': np.full((RFF, 1), np.float32(inp['t'][b]), np.float32),
        'label': _tf32(np.tile(np.asarray(inp['label'][b], np.float32)[:, None], (1, 2))),
        'freq': w['freq'], 'phase': w['phase'],
        'WxT': w['WxT'], 'sbias': w['sbias'],
        'WeT': w['WeT'], 'WlT': w['WlT'], 'WoT': w['WoT'],
        'frow': w['freq'].T.copy(), 'prow': w['phase'].T.copy(),
        'wurow': w['wurow'],
        'dw': w['dw'], 'hgT': w['hgT'], 'pcT': w['pcT'],
        'gfT': w['gfT'], 'gbT': w['gbT'], 'outT': w['outT'],
        'ones': np.ones((P, P), np.float32),
    }


def _build_program():
    import concourse.mybir as mybir
    import concourse.tile as tile
    from concourse import bacc

    F32 = mybir.dt.float32
    F32R = mybir.dt.float32r
    AF = mybir.ActivationFunctionType
    OP = mybir.AluOpType

    nc = bacc.Bacc(None)

    dp = nc.declare_dram_parameter
    x_in = dp("x", [DIM, W], F32R, isOutput=False)
    a_in = dp("a", [A_DIM, W], F32R, isOutput=False)
    mask_in = dp("mask", [P, W], F32, isOutput=False)
    tcol_in = dp("tcol", [RFF, 1], F32, isOutput=False)
    label_in = dp("label", [5, 2], F32R, isOutput=False)
    freq_in = dp("freq", [RFF, 1], F32, isOutput=False)
    phase_in = dp("phase", [RFF, 1], F32, isOutput=False)
    WxT_in = dp("WxT", [DIM, H_DIM], F32R, isOutput=False)
    sbias_in = dp("sbias", [H_DIM, 1], F32, isOutput=False)
    WeT_in = dp("WeT", [RFF, C_DIM], F32R, isOutput=False)
    WlT_in = dp("WlT", [5, C_DIM], F32R, isOutput=False)
    frow_in = dp("frow", [1, RFF], F32, isOutput=False)
    prow_in = dp("prow", [1, RFF], F32, isOutput=False)
    wurow_in = dp("wurow", [1, RFF], F32, isOutput=False)
    WoT_in = dp("WoT", [H_DIM, DIM], F32R, isOutput=False)
    dw_in = dp("dw", [H, DEPTH * 3], F32, isOutput=False)
    hgT_in = dp("hgT", [DEPTH, 384, 1024], F32R, isOutput=False)
    pcT_in = dp("pcT", [DEPTH, C_DIM, 1024], F32R, isOutput=False)
    gfT_in = dp("gfT", [DEPTH, H, 1024], F32R, isOutput=False)
    gbT_in = dp("gbT", [DEPTH, H, 1024], F32R, isOutput=False)
    outT_in = dp("outT", [DEPTH, H, H_DIM], F32R, isOutput=False)
    ones_in = dp("ones", [P, P], F32R, isOutput=False)
    o_out = dp("o", [DIM, 1024], F32, isOutput=True)
    u_out = dp("u", [1, 1], F32, isOutput=True)

    TWO_PI = float(2.0 * np.pi)
    ALPHA = float(0.7 / np.sqrt(0.58))

    with tile.TileContext(nc) as tc:
        with (
            tc.tile_pool(name="const", bufs=1) as cpool,
            tc.tile_pool(name="state", bufs=1) as spool,
            tc.tile_pool(name="wts", bufs=1) as wpool,
            tc.tile_pool(name="work", bufs=1) as wk,
            tc.tile_pool(name="mm", bufs=6, space="PSUM") as pmm,
            tc.tile_pool(name="vec", bufs=2, space="PSUM") as pvec,
        ):
            ones = cpool.tile([P, P], F32R, tag="ones", name="ones")
            nc.sync.dma_start(ones[:], ones_in[:])
            mask = cpool.tile([P, W], F32, tag="mask", name="mask")
            nc.sync.dma_start(mask[:], mask_in[:])
            eps_col = cpool.tile([P, 1], F32, tag="eps", name="eps_col")
            nc.vector.memset(eps_col[:], 1e-4)
            dw_t = []
            for k in range(4):
                t = cpool.tile([P, DEPTH * 3], F32, tag=f"dw{k}", name=f"dwt{k}")
                nc.sync.dma_start(t[:], dw_in[k * P:(k + 1) * P, :])
                dw_t.append(t)
            a_t = cpool.tile([P, W], F32R, tag="a", name="a_t")
            nc.sync.dma_start(a_t[:], a_in[:])
            sbias = []
            for m in range(2):
                t = cpool.tile([P, 1], F32, tag=f"sb{m}", name=f"sbias{m}")
                nc.sync.dma_start(t[:], sbias_in[m * P:(m + 1) * P, :])
                sbias.append(t)

            tcol = cpool.tile([RFF, 1], F32, tag="tcol", name="tcol")
            nc.sync.dma_start(tcol[:], tcol_in[:])
            freq = cpool.tile([RFF, 1], F32, tag="freq", name="freq")
            nc.sync.dma_start(freq[:], freq_in[:])
            phase = cpool.tile([RFF, 1], F32, tag="phase", name="phase")
            nc.sync.dma_start(phase[:], phase_in[:])
            label2 = cpool.tile([5, 2], F32R, tag="label", name="label2")
            nc.sync.dma_start(label2[:], label_in[:])
            WeT = wk.tile([RFF, C_DIM], F32R, tag="scr", bufs=4, name="WeT")
            nc.sync.dma_start(WeT[:], WeT_in[:])
            WlT = wk.tile([5, C_DIM], F32R, tag="scr", bufs=4, name="WlT")
            nc.sync.dma_start(WlT[:], WlT_in[:])
            frow = cpool.tile([1, RFF], F32, tag="frow", name="frow")
            nc.sync.dma_start(frow[:], frow_in[:])
            prow = cpool.tile([1, RFF], F32, tag="prow", name="prow")
            nc.sync.dma_start(prow[:], prow_in[:])
            wurow = cpool.tile([1, RFF], F32, tag="wurow", name="wurow")
            nc.sync.dma_start(wurow[:], wurow_in[:])

            xarg = wk.tile([RFF, 1], F32, tag="zsc", bufs=5, name="xarg")
            nc.vector.scalar_tensor_tensor(xarg[:], freq[:], tcol[:], phase[:],
                                           OP.mult, OP.add)
            m1 = wk.tile([RFF, 1], F32, tag="zsc", bufs=5, name="m1")
            nc.vector.tensor_scalar_add(m1[:], xarg[:], MAGIC)
            m2 = wk.tile([RFF, 1], F32, tag="zsc", bufs=5, name="m2")
            nc.vector.tensor_scalar_sub(m2[:], m1[:], MAGIC)
            red = wk.tile([RFF, 1], F32, tag="zsc", bufs=5, name="red")
            nc.vector.tensor_sub(red[:], xarg[:], m2[:])
            fsin = wk.tile([RFF, 2], F32R, tag="zsc", bufs=5, name="fsin")
            nc.scalar.activation(fsin[:, 0:1], red[:], AF.Sin, scale=TWO_PI)
            nc.scalar.activation(fsin[:, 1:2], red[:], AF.Sin, scale=TWO_PI)

            xargr = wk.tile([1, RFF], F32, tag="zsc", bufs=5, name="xargr")
            nc.vector.scalar_tensor_tensor(xargr[:], frow[:], tcol[:1, :], prow[:],
                                           OP.mult, OP.add)
            m1r = wk.tile([1, RFF], F32, tag="zsc", bufs=5, name="m1r")
            nc.vector.tensor_scalar_add(m1r[:], xargr[:], MAGIC)
            m2r = wk.tile([1, RFF], F32, tag="zsc", bufs=5, name="m2r")
            nc.vector.tensor_scalar_sub(m2r[:], m1r[:], MAGIC)
            redr = wk.tile([1, RFF], F32, tag="zsc", bufs=5, name="redr")
            nc.vector.tensor_sub(redr[:], xargr[:], m2r[:])
            sinr = wk.tile([1, RFF], F32, tag="zsc", bufs=5, name="sinr")
            nc.scalar.activation(sinr[:], redr[:], AF.Sin, scale=TWO_PI)
            usum = wk.tile([1, RFF], F32, tag="zsc", bufs=5, name="usum")
            nc.vector.tensor_mul(usum[:], wurow[:], sinr[:])
            usb = wk.tile([1, 1], F32, tag="zsc", bufs=5, name="usb")
            nc.vector.tensor_reduce(usb[:], usum[:], mybir.AxisListType.X,
                                    OP.add)
            nc.sync.dma_start(u_out[:], usb[:])

            c_t = []
            for m in range(2):
                cps = pvec.tile([P, 2], F32, tag="vps", name=f"cps{m}")
                nc.tensor.matmul(cps[:], WeT[:, m * P:(m + 1) * P], fsin[:],
                                 start=True, stop=False)
                nc.tensor.matmul(cps[:], WlT[:, m * P:(m + 1) * P], label2[:],
                                 start=False, stop=True)
                ct = cpool.tile([P, 2], F32R, tag=f"c{m}", name=f"c_t{m}")
                nc.scalar.activation(ct[:], cps[:], AF.Silu)
                c_t.append(ct)

            cm_all = cpool.tile([P, DEPTH * 8], F32, tag="cm", name="cm_all")
            for i in range(DEPTH):
                pc0 = wpool.tile([P, 1024], F32R, tag="wgf", bufs=4,
                                 name=f"pc0_{i}")
                nc.sync.dma_start(pc0[:], pcT_in[i, 0:P, :])
                pc1 = wpool.tile([P, 1024], F32R, tag="wgf", bufs=4,
                                 name=f"pc1_{i}")
                nc.sync.dma_start(pc1[:], pcT_in[i, P:C_DIM, :])
                for m in range(8):
                    cps = pvec.tile([P, 2], F32, tag="vps", name=f"cmps{i}_{m}")
                    nc.tensor.matmul(cps[:], pc0[:, m * P:(m + 1) * P], c_t[0][:],
                                     start=True, stop=False)
                    nc.tensor.matmul(cps[:], pc1[:, m * P:(m + 1) * P], c_t[1][:],
                                     start=False, stop=True)
                    nc.scalar.activation(cm_all[:, i * 8 + m:i * 8 + m + 1],
                                         cps[:, 0:1], AF.Copy, bias=1.0)

            xt = wk.tile([DIM, W], F32R, tag="scr", bufs=4, name="xt")
            nc.sync.dma_start(xt[:], x_in[:])
            WxT = wk.tile([DIM, H_DIM], F32R, tag="scr", bufs=4, name="WxT")
            nc.sync.dma_start(WxT[:], WxT_in[:])
            h_t = [spool.tile([P, W], F32, tag=f"h{m}", name=f"h_t{m}")
                   for m in range(2)]
            for n in range(NCH):
                sl = slice(n * CH, (n + 1) * CH)
                for m in range(2):
                    ps = pmm.tile([P, CH], F32, tag="ps", name=f"st_ps{n}_{m}")
                    nc.tensor.matmul(ps[:], WxT[:, m * P:(m + 1) * P], xt[:, sl],
                                     start=True, stop=True)
                    nc.vector.scalar_tensor_tensor(h_t[m][:, sl], mask[:, sl],
                                                   sbias[m][:], ps[:],
                                                   OP.mult, OP.add)

            for i in range(DEPTH):
                hg_w = []
                for kk in range(3):
                    t = wpool.tile([P, 1024], F32R, tag="whg", bufs=3,
                                   name=f"hgw{i}_{kk}")
                    nc.sync.dma_start(t[:], hgT_in[i, kk * P:(kk + 1) * P, :])
                    hg_w.append(t)
                gf_w = []
                gb_w = []
                for kk in range(4):
                    t = wpool.tile([P, 1024], F32R, tag="wgf", bufs=4,
                                   name=f"gfw{i}_{kk}")
                    nc.sync.dma_start(t[:], gfT_in[i, kk * P:(kk + 1) * P, :])
                    gf_w.append(t)
                for kk in range(4):
                    t = wpool.tile([P, 1024], F32R, tag="wgb", bufs=4,
                                   name=f"gbw{i}_{kk}")
                    nc.sync.dma_start(t[:], gbT_in[i, kk * P:(kk + 1) * P, :])
                    gb_w.append(t)
                out_w = []
                for kk in range(4):
                    t = wpool.tile([P, H_DIM], F32R, tag="wout", bufs=4,
                                   name=f"outw{i}_{kk}")
                    nc.sync.dma_start(t[:], outT_in[i, kk * P:(kk + 1) * P, :])
                    out_w.append(t)

                hsq = []
                for m in range(2):
                    t = wk.tile([P, W], F32R, tag="scr", bufs=4,
                                name=f"hsq{i}_{m}")
                    nc.scalar.activation(t[:], h_t[m][:], AF.Square)
                    hsq.append(t)
                rinv = wk.tile([P, W], F32, tag="rinv", name=f"rinv{i}")
                for n in range(NCH):
                    sl = slice(n * CH, (n + 1) * CH)
                    ps = pmm.tile([P, CH], F32, tag="ps", name=f"pn_ps{i}_{n}")
                    nc.tensor.matmul(ps[:], ones[:], hsq[0][:, sl],
                                     start=True, stop=False)
                    nc.tensor.matmul(ps[:], ones[:], hsq[1][:, sl],
                                     start=False, stop=True)
                    tsd = wk.tile([P, CH], F32, tag="zsc", bufs=5,
                                  name=f"tsd{i}_{n}")
                    nc.scalar.activation(tsd[:], ps[:], AF.Sqrt,
                                         bias=eps_col[:], scale=1.0 / H_DIM)
                    nc.vector.reciprocal_approx_fast(rinv[:, sl], tsd[:])
                hsil = []
                for m in range(2):
                    t = wk.tile([P, W], F32R, tag="sca", bufs=4,
                                name=f"hsil{i}_{m}")
                    hsil.append(t)
                xn = [wk.tile([P, W], F32, tag="scr", bufs=4, name=f"xn{i}_{m}")
                      for m in range(2)]
                for n in range(NCH):
                    sl = slice(n * CH, (n + 1) * CH)
                    for m in range(2):
                        nc.vector.tensor_mul(xn[m][:, sl], h_t[m][:, sl],
                                             rinv[:, sl])
                        nc.scalar.activation(hsil[m][:, sl], xn[m][:, sl],
                                             AF.Silu)

                rhs3 = [hsil[0], hsil[1], a_t]
                halo = [wk.tile([P, W + 2], F32, tag="hab", bufs=4,
                                name=f"halo{i}_{k}") for k in range(4)]
                for t in halo:
                    nc.vector.memset(t[:, 0:1], 0.0)
                    nc.vector.memset(t[:, W + 1:W + 2], 0.0)
                gsil = [wk.tile([P, W], F32, tag="gsil", bufs=4,
                                name=f"gsil{i}_{k}") for k in range(4)]
                for n in range(NCH):
                    sl = slice(n * CH, (n + 1) * CH)
                    for m in range(8):
                        ps = pmm.tile([P, CH], F32, tag="ps",
                                      name=f"hg_ps{i}_{n}_{m}")
                        for kk in range(3):
                            nc.tensor.matmul(ps[:],
                                             hg_w[kk][:, m * P:(m + 1) * P],
                                             rhs3[kk][:, sl],
                                             start=(kk == 0), stop=(kk == 2))
                        if m < 4:
                            nc.scalar.activation(
                                halo[m][:, 1 + n * CH:1 + (n + 1) * CH], ps[:],
                                AF.Copy)
                        else:
                            nc.scalar.activation(
                                gsil[m - 4][:, sl], ps[:], AF.Silu,
                                scale=cm_all[:, i * 8 + m:i * 8 + m + 1])

                h1a = [wk.tile([P, W], F32R, tag="h1a", bufs=4,
                               name=f"h1a{i}_{k}") for k in range(4)]
                t2s = [wk.tile([P, W], F32, tag="scr", bufs=4,
                               name=f"dwt2_{i}_{k}") for k in range(4)]
                for n in range(NCH):
                    sl = slice(n * CH, (n + 1) * CH)
                    for k in range(4):
                        c0 = dw_t[k][:, i * 3 + 0:i * 3 + 1]
                        c1 = dw_t[k][:, i * 3 + 1:i * 3 + 2]
                        c2 = dw_t[k][:, i * 3 + 2:i * 3 + 3]
                        lo = n * CH
                        m1_ = wk.tile([P, CH], F32, tag="zsc", bufs=5,
                                      name=f"dwm{i}_{n}_{k}")
                        nc.scalar.activation(m1_[:], halo[k][:, 1 + lo:1 + lo + CH],
                                             AF.Copy, scale=c1)
                        t1 = wk.tile([P, CH], F32, tag="zsc", bufs=5,
                                     name=f"dwt1_{i}_{n}_{k}")
                        nc.vector.scalar_tensor_tensor(
                            t1[:], halo[k][:, lo:lo + CH], c0, m1_[:],
                            OP.mult, OP.add)
                        nc.vector.scalar_tensor_tensor(
                            t2s[k][:, sl], halo[k][:, 2 + lo:2 + lo + CH], c2,
                            t1[:], OP.mult, OP.add)
                for k in range(4):
                    nc.vector.tensor_mul(t2s[k][:, 0:PAD + 1],
                                         t2s[k][:, 0:PAD + 1],
                                         mask[:, 0:PAD + 1])
                    nc.vector.tensor_mul(t2s[k][:, W - PAD - 1:W],
                                         t2s[k][:, W - PAD - 1:W],
                                         mask[:, W - PAD - 1:W])
                    nc.scalar.activation(h1a[k][:], t2s[k][:], AF.Silu,
                                         scale=cm_all[:, i * 8 + k:i * 8 + k + 1])

                def gru_pass(g_w, reverse, li):
                    bco, aco = [], []
                    for k in range(4):
                        bco.append(wk.tile([P, W], F32, tag="bco", bufs=4,
                                           name=f"bco{li}_{reverse}_{k}"))
                        aco.append(wk.tile([P, W], F32, tag="sca", bufs=4,
                                           name=f"aco{li}_{reverse}_{k}"))
                    for n in range(NCH):
                        sl = slice(n * CH, (n + 1) * CH)
                        zs_ch = [None] * 4
                        for m in range(8):
                            ps = pmm.tile([P, CH], F32, tag="ps",
                                          name=f"g_ps{li}_{reverse}_{n}_{m}")
                            for kk in range(4):
                                nc.tensor.matmul(ps[:],
                                                 g_w[kk][:, m * P:(m + 1) * P],
                                                 h1a[kk][:, sl],
                                                 start=(kk == 0), stop=(kk == 3))
                            if m < 4:
                                zc = wk.tile([P, CH], F32, tag="zsc", bufs=5,
                                             name=f"zc{li}_{reverse}_{n}_{m}")
                                nc.scalar.activation(zc[:], ps[:], AF.Sigmoid)
                                zs_ch[m] = zc
                            else:
                                k = m - 4
                                nc.vector.tensor_mul(bco[k][:, sl],
                                                     zs_ch[k][:], ps[:])
                                nc.gpsimd.tensor_scalar(aco[k][:, sl],
                                                        zs_ch[k][:], -1.0, 1.0,
                                                        OP.mult, OP.add)
                    outs = []
                    for k in range(4):
                        o_ = wk.tile([P, W], F32,
                                     tag=("hab" if reverse else "fwd"), bufs=4,
                                     name=f"scan{li}_{reverse}_{k}")
                        if reverse:
                            nc.vector.tensor_tensor_scan(
                                o_[:, ::-1], aco[k][:, ::-1], bco[k][:, ::-1],
                                0.0, OP.mult, OP.add)
                        else:
                            nc.vector.tensor_tensor_scan(
                                o_[:], aco[k][:], bco[k][:], 0.0,
                                OP.mult, OP.add)
                        outs.append(o_)
                    return outs

                fwd = gru_pass(gf_w, False, i)
                bwd = gru_pass(gb_w, True, i)

                prod = []
                for k in range(4):
                    h1s = wk.tile([P, W], F32, tag="sca", bufs=4,
                                  name=f"h1s{i}_{k}")
                    nc.gpsimd.tensor_add(h1s[:], fwd[k][:], bwd[k][:])
                    pr = wk.tile([P, W], F32R, tag="bco", bufs=4,
                                 name=f"prod{i}_{k}")
                    nc.vector.tensor_mul(pr[:], h1s[:], gsil[k][:])
                    prod.append(pr)
                xn2 = []
                for m in range(2):
                    t = wk.tile([P, W], F32, tag="scr", bufs=4, name=f"xn2_{i}_{m}")
                    nc.vector.tensor_mul(t[:], h_t[m][:], rinv[:])
                    xn2.append(t)
                for n in range(NCH):
                    sl = slice(n * CH, (n + 1) * CH)
                    for m in range(2):
                        ps = pmm.tile([P, CH], F32, tag="ps",
                                      name=f"o_ps{i}_{n}_{m}")
                        for kk in range(4):
                            nc.tensor.matmul(ps[:],
                                             out_w[kk][:, m * P:(m + 1) * P],
                                             prod[kk][:, sl],
                                             start=(kk == 0), stop=(kk == 3))
                        nc.vector.scalar_tensor_tensor(
                            h_t[m][:, sl], xn2[m][:, sl], ALPHA, ps[:],
                            OP.mult, OP.add)

            Wo_t = []
            for m in range(2):
                t = cpool.tile([P, DIM], F32R, tag=f"Wo{m}", name=f"Wo_t{m}")
                nc.sync.dma_start(t[:], WoT_in[m * P:(m + 1) * P, :])
                Wo_t.append(t)
            h_r = []
            for m in range(2):
                t = wk.tile([P, W], F32R, tag="scr", bufs=4, name=f"h_r{m}")
                nc.scalar.activation(t[:], h_t[m][:], AF.Copy)
                h_r.append(t)
            o_sb = wk.tile([DIM, 1024], F32, tag="scr", bufs=4, name="o_sb")
            for (c0, cw) in ((0, 384), (384, 384), (768, 256)):
                ps = pmm.tile([DIM, 384], F32, tag="ps", name=f"head_ps{c0}")
                for m in range(2):
                    nc.tensor.matmul(ps[:, :cw], Wo_t[m][:],
                                     h_r[m][:, PAD + c0:PAD + c0 + cw],
                                     start=(m == 0), stop=(m == 1))
                nc.scalar.activation(o_sb[:, c0:c0 + cw], ps[:, :cw], AF.Copy)
            nc.sync.dma_start(o_out[:], o_sb[:])

    nc.compile()
    return nc


def _get_prog():
    global _PROG
    if _PROG is None:
        _PROG = _build_program()
    return _PROG


def kernel(**inputs):
    from concourse.bass_utils import run_bass_kernel_spmd

    w = _prep_weights(inputs)
    in_maps = []
    for b in range(B):
        for half in (0, 1):
            in_maps.append(_core_inputs(inputs, w, b, half))
    nc = _get_prog()
    res = run_bass_kernel_spmd(nc, in_maps, list(range(8)))
    o = np.zeros((B, DIM, L), np.float32)
    u = np.zeros((B,), np.float32)
    for ci, r in enumerate(res.results):
        b, half = divmod(ci, 2)
        o[b, :, half * 1024:(half + 1) * 1024] = r["o"]
        if half == 0:
            u[b] = r["u"][0, 0]
    return o, u


# revision 9
# speedup vs baseline: 1.9215x; 1.2596x over previous
import numpy as np

B, L = 4, 2048
DIM, A_DIM, H_DIM, DEPTH = 32, 128, 256, 8
RFF, C_DIM = 64, 256
H = 512
PAD = 32
W = PAD + 1024 + PAD
CHUNKS = ((0, 384), (384, 384), (768, 320))
NCH = 3
CH = 384
P = 128
MAGIC = float(12582912.0)

_PROG = None


def _tf32(x):
    x = np.ascontiguousarray(x, np.float32)
    u = x.view(np.uint32)
    r = ((u.astype(np.uint64) + 0x1000 + ((u >> 13) & 1)) & ~np.uint64(0x1FFF))
    return r.astype(np.uint32).view(np.float32)


def _wnorm(w):
    n = np.sqrt(np.sum(w * w, axis=tuple(range(1, w.ndim)), keepdims=True))
    return (w / (n + 1e-8)).astype(np.float32)


def _prep_weights(inp):
    f32 = lambda x: np.asarray(x, np.float32)
    out = {}
    ca = np.float32(0.5 * np.sqrt(66.0 / 32.0))
    cb = np.float32(0.5 * np.sqrt(66.0 / 1.0))
    Wh = _wnorm(f32(inp['W_h']))
    out['WxT'] = _tf32((Wh[:, :DIM] * ca).T)
    out['sbias'] = np.ascontiguousarray(Wh[:, DIM] * cb)[:, None]

    out['freq'] = np.ascontiguousarray(f32(inp['rff_freq']))[:, None]
    out['phase'] = np.ascontiguousarray(f32(inp['rff_phase']) + 0.25)[:, None]
    SQ2 = np.float32(np.sqrt(2.0))
    out['WeT'] = _tf32(_wnorm(f32(inp['W_e'])).T)
    out['WlT'] = _tf32((_wnorm(f32(inp['W_label'])) / SQ2).T)
    out['wurow'] = np.ascontiguousarray((_wnorm(f32(inp['W_u'])) * SQ2))

    ca2 = np.float32(0.5 * np.sqrt(3.0))
    cb2 = np.float32(0.5 * np.sqrt(6.0))
    IS = np.float32(1.0 / 0.596)
    hgT, pcT, gfT, gbT, outT, dw = [], [], [], [], [], []
    for i in range(DEPTH):
        hg = _wnorm(f32(inp['hg_W'][i]))
        hg_eff = np.concatenate([hg[:, :H_DIM] * (ca2 * IS), hg[:, H_DIM:] * cb2], 1)
        hgT.append(_tf32(hg_eff.T))
        pc = _wnorm(f32(inp['projc_W'][i])) * (f32(inp['projc_gain'][i]) * IS)
        pcT.append(_tf32(pc.T))
        gfT.append(_tf32((_wnorm(f32(inp['gruf_W'][i])) * IS).T))
        gbT.append(_tf32((_wnorm(f32(inp['grub_W'][i])) * IS).T))
        sc = np.float32(0.3 / (np.sqrt(0.58) * np.sqrt(2.0) * 0.596))
        outT.append(_tf32((_wnorm(f32(inp['out_W'][i])) * sc).T))
        dw.append(_wnorm(f32(inp['dw_W'][i]))[:, 0, :])
    out['hgT'] = np.ascontiguousarray(np.stack(hgT))
    out['pcT'] = np.ascontiguousarray(np.stack(pcT))
    out['gfT'] = np.ascontiguousarray(np.stack(gfT))
    out['gbT'] = np.ascontiguousarray(np.stack(gbT))
    out['outT'] = np.ascontiguousarray(np.stack(outT))
    out['dw'] = np.ascontiguousarray(np.stack(dw, axis=1).reshape(H, DEPTH * 3))
    out['WoT'] = _tf32((_wnorm(f32(inp['Wo'])) * f32(inp['out_gain'])).T)
    return out


def _core_inputs(inp, w, b, half):
    g0 = half * 1024 - PAD
    x_pad = np.zeros((DIM, W), np.float32)
    a_pad = np.zeros((A_DIM, W), np.float32)
    mask = np.zeros((1, W), np.float32)
    lo, hi = max(g0, 0), min(g0 + W, L)
    x_pad[:, lo - g0:hi - g0] = np.asarray(inp['x'][b], np.float32)[:, lo:hi]
    a_pad[:, lo - g0:hi - g0] = np.asarray(inp['a'][b], np.float32)[:, lo:hi]
    mask[:, lo - g0:hi - g0] = 1.0
    return {
        'x': _tf32(x_pad),
        'a': _tf32(a_pad),
        'mask': np.ascontiguousarray(np.broadcast_to(mask, (P, W))),
        'tcol': np.full((RFF, 1), np.float32(inp['t'][b]), np.float32),
        'label': _tf32(np.tile(np.asarray(inp['label'][b], np.float32)[:, None], (1, 2))),
        'freq': w['freq'], 'phase': w['phase'],
        'WxT': w['WxT'], 'sbias': w['sbias'],
        'WeT': w['WeT'], 'WlT': w['WlT'], 'WoT': w['WoT'],
        'frow': w['freq'].T.copy(), 'prow': w['phase'].T.copy(),
        'wurow': w['wurow'],
        'dw': w['dw'], 'hgT': w['hgT'], 'pcT': w['pcT'],
        'gfT': w['gfT'], 'gbT': w['gbT'], 'outT': w['outT'],
        'ones': np.ones((P, P), np.float32),
    }


def _build_program():
    import concourse.mybir as mybir
    import concourse.tile as tile
    from concourse import bacc

    F32 = mybir.dt.float32
    F32R = mybir.dt.float32r
    AF = mybir.ActivationFunctionType
    OP = mybir.AluOpType

    nc = bacc.Bacc(None)

    dp = nc.declare_dram_parameter
    x_in = dp("x", [DIM, W], F32R, isOutput=False)
    a_in = dp("a", [A_DIM, W], F32R, isOutput=False)
    mask_in = dp("mask", [P, W], F32, isOutput=False)
    tcol_in = dp("tcol", [RFF, 1], F32, isOutput=False)
    label_in = dp("label", [5, 2], F32R, isOutput=False)
    freq_in = dp("freq", [RFF, 1], F32, isOutput=False)
    phase_in = dp("phase", [RFF, 1], F32, isOutput=False)
    WxT_in = dp("WxT", [DIM, H_DIM], F32R, isOutput=False)
    sbias_in = dp("sbias", [H_DIM, 1], F32, isOutput=False)
    WeT_in = dp("WeT", [RFF, C_DIM], F32R, isOutput=False)
    WlT_in = dp("WlT", [5, C_DIM], F32R, isOutput=False)
    frow_in = dp("frow", [1, RFF], F32, isOutput=False)
    prow_in = dp("prow", [1, RFF], F32, isOutput=False)
    wurow_in = dp("wurow", [1, RFF], F32, isOutput=False)
    WoT_in = dp("WoT", [H_DIM, DIM], F32R, isOutput=False)
    dw_in = dp("dw", [H, DEPTH * 3], F32, isOutput=False)
    hgT_in = dp("hgT", [DEPTH, 384, 1024], F32R, isOutput=False)
    pcT_in = dp("pcT", [DEPTH, C_DIM, 1024], F32R, isOutput=False)
    gfT_in = dp("gfT", [DEPTH, H, 1024], F32R, isOutput=False)
    gbT_in = dp("gbT", [DEPTH, H, 1024], F32R, isOutput=False)
    outT_in = dp("outT", [DEPTH, H, H_DIM], F32R, isOutput=False)
    ones_in = dp("ones", [P, P], F32R, isOutput=False)
    o_out = dp("o", [DIM, 1024], F32, isOutput=True)
    u_out = dp("u", [1, 1], F32, isOutput=True)

    TWO_PI = float(2.0 * np.pi)
    ALPHA = float(0.7 / np.sqrt(0.58))

    with tile.TileContext(nc) as tc:
        with (
            tc.tile_pool(name="const", bufs=1) as cpool,
            tc.tile_pool(name="state", bufs=1) as spool,
            tc.tile_pool(name="wts", bufs=1) as wpool,
            tc.tile_pool(name="work", bufs=1) as wk,
            tc.tile_pool(name="mm", bufs=6, space="PSUM") as pmm,
            tc.tile_pool(name="vec", bufs=2, space="PSUM") as pvec,
        ):
            ones = cpool.tile([P, P], F32R, tag="ones", name="ones")
            nc.sync.dma_start(ones[:], ones_in[:])
            mask = cpool.tile([P, W], F32, tag="mask", name="mask")
            nc.sync.dma_start(mask[:], mask_in[:])
            eps_col = cpool.tile([P, 1], F32, tag="eps", name="eps_col")
            nc.vector.memset(eps_col[:], 1e-4)
            dw_t = []
            for k in range(4):
                t = cpool.tile([P, DEPTH * 3], F32, tag=f"dw{k}", name=f"dwt{k}")
                nc.sync.dma_start(t[:], dw_in[k * P:(k + 1) * P, :])
                dw_t.append(t)
            a_t = cpool.tile([P, W], F32R, tag="a", name="a_t")
            nc.sync.dma_start(a_t[:], a_in[:])
            sbias = []
            for m in range(2):
                t = cpool.tile([P, 1], F32, tag=f"sb{m}", name=f"sbias{m}")
                nc.sync.dma_start(t[:], sbias_in[m * P:(m + 1) * P, :])
                sbias.append(t)

            tcol = cpool.tile([RFF, 1], F32, tag="tcol", name="tcol")
            nc.sync.dma_start(tcol[:], tcol_in[:])
            freq = cpool.tile([RFF, 1], F32, tag="freq", name="freq")
            nc.sync.dma_start(freq[:], freq_in[:])
            phase = cpool.tile([RFF, 1], F32, tag="phase", name="phase")
            nc.sync.dma_start(phase[:], phase_in[:])
            label2 = cpool.tile([5, 2], F32R, tag="label", name="label2")
            nc.sync.dma_start(label2[:], label_in[:])
            WeT = wk.tile([RFF, C_DIM], F32R, tag="scr", bufs=4, name="WeT")
            nc.sync.dma_start(WeT[:], WeT_in[:])
            WlT = wk.tile([5, C_DIM], F32R, tag="scr", bufs=4, name="WlT")
            nc.sync.dma_start(WlT[:], WlT_in[:])
            frow = cpool.tile([1, RFF], F32, tag="frow", name="frow")
            nc.sync.dma_start(frow[:], frow_in[:])
            prow = cpool.tile([1, RFF], F32, tag="prow", name="prow")
            nc.sync.dma_start(prow[:], prow_in[:])
            wurow = cpool.tile([1, RFF], F32, tag="wurow", name="wurow")
            nc.sync.dma_start(wurow[:], wurow_in[:])

            xarg = wk.tile([RFF, 1], F32, tag="zsc", bufs=5, name="xarg")
            nc.vector.scalar_tensor_tensor(xarg[:], freq[:], tcol[:], phase[:],
                                           OP.mult, OP.add)
            m1 = wk.tile([RFF, 1], F32, tag="zsc", bufs=5, name="m1")
            nc.vector.tensor_scalar_add(m1[:], xarg[:], MAGIC)
            m2 = wk.tile([RFF, 1], F32, tag="zsc", bufs=5, name="m2")
            nc.vector.tensor_scalar_sub(m2[:], m1[:], MAGIC)
            red = wk.tile([RFF, 1], F32, tag="zsc", bufs=5, name="red")
            nc.vector.tensor_sub(red[:], xarg[:], m2[:])
            fsin = wk.tile([RFF, 2], F32R, tag="zsc", bufs=5, name="fsin")
            nc.scalar.activation(fsin[:, 0:1], red[:], AF.Sin, scale=TWO_PI)
            nc.scalar.activation(fsin[:, 1:2], red[:], AF.Sin, scale=TWO_PI)

            xargr = wk.tile([1, RFF], F32, tag="zsc", bufs=5, name="xargr")
            nc.vector.scalar_tensor_tensor(xargr[:], frow[:], tcol[:1, :], prow[:],
                                           OP.mult, OP.add)
            m1r = wk.tile([1, RFF], F32, tag="zsc", bufs=5, name="m1r")
            nc.vector.tensor_scalar_add(m1r[:], xargr[:], MAGIC)
            m2r = wk.tile([1, RFF], F32, tag="zsc", bufs=5, name="m2r")
            nc.vector.tensor_scalar_sub(m2r[:], m1r[:], MAGIC)
            redr = wk.tile([1, RFF], F32, tag="zsc", bufs=5, name="redr")
            nc.vector.tensor_sub(redr[:], xargr[:], m2r[:])
            sinr = wk.tile([1, RFF], F32, tag="zsc", bufs=5, name="sinr")
            nc.scalar.activation(sinr[:], redr[:], AF.Sin, scale=TWO_PI)
            usum = wk.tile([1, RFF], F32, tag="zsc", bufs=5, name="usum")
            nc.vector.tensor_mul(usum[:], wurow[:], sinr[:])
            usb = wk.tile([1, 1], F32, tag="zsc", bufs=5, name="usb")
            nc.vector.tensor_reduce(usb[:], usum[:], mybir.AxisListType.X,
                                    OP.add)
            nc.sync.dma_start(u_out[:], usb[:])

            c_t = []
            for m in range(2):
                cps = pvec.tile([P, 2], F32, tag="vps", name=f"cps{m}")
                nc.tensor.matmul(cps[:], WeT[:, m * P:(m + 1) * P], fsin[:],
                                 start=True, stop=False)
                nc.tensor.matmul(cps[:], WlT[:, m * P:(m + 1) * P], label2[:],
                                 start=False, stop=True)
                ct = cpool.tile([P, 2], F32R, tag=f"c{m}", name=f"c_t{m}")
                nc.scalar.activation(ct[:], cps[:], AF.Silu)
                c_t.append(ct)

            cm_all = cpool.tile([P, DEPTH * 8], F32, tag="cm", name="cm_all")
            for i in range(DEPTH):
                pc0 = wpool.tile([P, 1024], F32R, tag="wgf", bufs=4,
                                 name=f"pc0_{i}")
                nc.sync.dma_start(pc0[:], pcT_in[i, 0:P, :])
                pc1 = wpool.tile([P, 1024], F32R, tag="wgf", bufs=4,
                                 name=f"pc1_{i}")
                nc.sync.dma_start(pc1[:], pcT_in[i, P:C_DIM, :])
                for m in range(8):
                    cps = pvec.tile([P, 2], F32, tag="vps", name=f"cmps{i}_{m}")
                    nc.tensor.matmul(cps[:], pc0[:, m * P:(m + 1) * P], c_t[0][:],
                                     start=True, stop=False)
                    nc.tensor.matmul(cps[:], pc1[:, m * P:(m + 1) * P], c_t[1][:],
                                     start=False, stop=True)
                    nc.scalar.activation(cm_all[:, i * 8 + m:i * 8 + m + 1],
                                         cps[:, 0:1], AF.Copy, bias=1.0)

            xt = wk.tile([DIM, W], F32R, tag="scr", bufs=4, name="xt")
            nc.sync.dma_start(xt[:], x_in[:])
            WxT = wk.tile([DIM, H_DIM], F32R, tag="scr", bufs=4, name="WxT")
            nc.sync.dma_start(WxT[:], WxT_in[:])
            h_t = [spool.tile([P, W], F32, tag=f"h{m}", name=f"h_t{m}")
                   for m in range(2)]
            for (lo, cw) in CHUNKS:
                sl = slice(lo, lo + cw)
                for m in range(2):
                    ps = pmm.tile([P, CH], F32, tag="ps", name=f"st_ps{lo}_{m}")
                    nc.tensor.matmul(ps[:, :cw], WxT[:, m * P:(m + 1) * P],
                                     xt[:, sl], start=True, stop=True)
                    nc.vector.scalar_tensor_tensor(h_t[m][:, sl], mask[:, sl],
                                                   sbias[m][:], ps[:, :cw],
                                                   OP.mult, OP.add)

            for i in range(DEPTH):
                hg_w = []
                for kk in range(3):
                    t = wpool.tile([P, 1024], F32R, tag="whg", bufs=3,
                                   name=f"hgw{i}_{kk}")
                    nc.sync.dma_start(t[:], hgT_in[i, kk * P:(kk + 1) * P, :])
                    hg_w.append(t)
                gf_w = []
                gb_w = []
                for kk in range(4):
                    t = wpool.tile([P, 1024], F32R, tag="wgf", bufs=4,
                                   name=f"gfw{i}_{kk}")
                    nc.sync.dma_start(t[:], gfT_in[i, kk * P:(kk + 1) * P, :])
                    gf_w.append(t)
                for kk in range(4):
                    t = wpool.tile([P, 1024], F32R, tag="wgb", bufs=4,
                                   name=f"gbw{i}_{kk}")
                    nc.sync.dma_start(t[:], gbT_in[i, kk * P:(kk + 1) * P, :])
                    gb_w.append(t)
                out_w = []
                for kk in range(4):
                    t = wpool.tile([P, H_DIM], F32R, tag="wout", bufs=4,
                                   name=f"outw{i}_{kk}")
                    nc.sync.dma_start(t[:], outT_in[i, kk * P:(kk + 1) * P, :])
                    out_w.append(t)

                hsq = [wk.tile([P, W], F32R, tag="scr", bufs=4,
                               name=f"hsq{i}_{m}") for m in range(2)]
                rinv = wk.tile([P, W], F32, tag="rinv", name=f"rinv{i}")
                hsil = [wk.tile([P, W], F32R, tag="sca", bufs=4,
                                name=f"hsil{i}_{m}") for m in range(2)]
                xn = [wk.tile([P, W], F32, tag="scr", bufs=4, name=f"xn{i}_{m}")
                      for m in range(2)]
                for (lo, cw) in CHUNKS:
                    sl = slice(lo, lo + cw)
                    for m in range(2):
                        nc.scalar.activation(hsq[m][:, sl], h_t[m][:, sl],
                                             AF.Square)
                    ps = pmm.tile([P, CH], F32, tag="ps", name=f"pn_ps{i}_{lo}")
                    nc.tensor.matmul(ps[:, :cw], ones[:], hsq[0][:, sl],
                                     start=True, stop=False)
                    nc.tensor.matmul(ps[:, :cw], ones[:], hsq[1][:, sl],
                                     start=False, stop=True)
                    tsd = wk.tile([P, CH], F32, tag="zsc", bufs=5,
                                  name=f"tsd{i}_{lo}")
                    nc.scalar.activation(tsd[:, :cw], ps[:, :cw], AF.Sqrt,
                                         bias=eps_col[:], scale=1.0 / H_DIM)
                    nc.vector.reciprocal_approx_fast(rinv[:, sl], tsd[:, :cw])
                    for m in range(2):
                        nc.vector.tensor_mul(xn[m][:, sl], h_t[m][:, sl],
                                             rinv[:, sl])
                        nc.scalar.activation(hsil[m][:, sl], xn[m][:, sl],
                                             AF.Silu)
                xn2 = []
                for m in range(2):
                    t = wk.tile([P, W], F32, tag="scr", bufs=4,
                                name=f"xn2_{i}_{m}")
                    nc.gpsimd.tensor_mul(t[:], h_t[m][:], rinv[:])
                    xn2.append(t)

                rhs3 = [hsil[0], hsil[1], a_t]
                halo = [wk.tile([P, W + 2], F32, tag="hab", bufs=4,
                                name=f"halo{i}_{k}") for k in range(4)]
                for t in halo:
                    nc.vector.memset(t[:, 0:1], 0.0)
                    nc.vector.memset(t[:, W + 1:W + 2], 0.0)
                gsil = [wk.tile([P, W], F32, tag="gsil", bufs=4,
                                name=f"gsil{i}_{k}") for k in range(4)]
                h1a = [wk.tile([P, W], F32R, tag="h1a", bufs=4,
                               name=f"h1a{i}_{k}") for k in range(4)]
                t2s = [wk.tile([P, W], F32, tag="scr", bufs=4,
                               name=f"dwt2_{i}_{k}") for k in range(4)]

                def dw_chain(k):
                    c0 = dw_t[k][:, i * 3 + 0:i * 3 + 1]
                    c1 = dw_t[k][:, i * 3 + 1:i * 3 + 2]
                    c2 = dw_t[k][:, i * 3 + 2:i * 3 + 3]
                    for (lo, cw) in CHUNKS:
                        m1_ = wk.tile([P, CH], F32, tag="zsc", bufs=5,
                                      name=f"dwm{i}_{lo}_{k}")
                        nc.scalar.activation(m1_[:, :cw],
                                             halo[k][:, 1 + lo:1 + lo + cw],
                                             AF.Copy, scale=c1)
                        t1 = wk.tile([P, CH], F32, tag="zsc", bufs=5,
                                     name=f"dwt1_{i}_{lo}_{k}")
                        nc.vector.scalar_tensor_tensor(
                            t1[:, :cw], halo[k][:, lo:lo + cw], c0, m1_[:, :cw],
                            OP.mult, OP.add)
                        nc.vector.scalar_tensor_tensor(
                            t2s[k][:, lo:lo + cw],
                            halo[k][:, 2 + lo:2 + lo + cw], c2,
                            t1[:, :cw], OP.mult, OP.add)
                    nc.vector.tensor_mul(t2s[k][:, 0:PAD + 1],
                                         t2s[k][:, 0:PAD + 1],
                                         mask[:, 0:PAD + 1])
                    nc.vector.tensor_mul(t2s[k][:, W - PAD - 1:W],
                                         t2s[k][:, W - PAD - 1:W],
                                         mask[:, W - PAD - 1:W])
                    nc.scalar.activation(h1a[k][:], t2s[k][:], AF.Silu,
                                         scale=cm_all[:, i * 8 + k:i * 8 + k + 1])

                for m in range(8):
                    for (lo, cw) in CHUNKS:
                        sl = slice(lo, lo + cw)
                        ps = pmm.tile([P, CH], F32, tag="ps",
                                      name=f"hg_ps{i}_{lo}_{m}")
                        for kk in range(3):
                            nc.tensor.matmul(ps[:, :cw],
                                             hg_w[kk][:, m * P:(m + 1) * P],
                                             rhs3[kk][:, sl],
                                             start=(kk == 0), stop=(kk == 2))
                        if m < 4:
                            nc.scalar.activation(
                                halo[m][:, 1 + lo:1 + lo + cw], ps[:, :cw],
                                AF.Copy)
                        else:
                            nc.scalar.activation(
                                gsil[m - 4][:, sl], ps[:, :cw], AF.Silu,
                                scale=cm_all[:, i * 8 + m:i * 8 + m + 1])
                    if m < 4:
                        dw_chain(m)

                def gru_pass(g_w, reverse, li):
                    bco = [wk.tile([P, W], F32, tag="bco", bufs=4,
                                   name=f"bco{li}_{reverse}_{k}")
                           for k in range(4)]
                    aco = [wk.tile([P, W], F32, tag="sca", bufs=4,
                                   name=f"aco{li}_{reverse}_{k}")
                           for k in range(4)]
                    outs = []
                    for k in range(4):
                        zs_ch = []
                        for (lo, cw) in CHUNKS:
                            ps = pmm.tile([P, CH], F32, tag="ps",
                                          name=f"gz_ps{li}_{reverse}_{lo}_{k}")
                            for kk in range(4):
                                nc.tensor.matmul(ps[:, :cw],
                                                 g_w[kk][:, k * P:(k + 1) * P],
                                                 h1a[kk][:, lo:lo + cw],
                                                 start=(kk == 0), stop=(kk == 3))
                            zc = wk.tile([P, CH], F32, tag="zsc", bufs=5,
                                         name=f"zc{li}_{reverse}_{lo}_{k}")
                            nc.scalar.activation(zc[:, :cw], ps[:, :cw],
                                                 AF.Sigmoid)
                            zs_ch.append(zc)
                        for ci_, (lo, cw) in enumerate(CHUNKS):
                            sl = slice(lo, lo + cw)
                            m = k + 4
                            ps = pmm.tile([P, CH], F32, tag="ps",
                                          name=f"gh_ps{li}_{reverse}_{lo}_{k}")
                            for kk in range(4):
                                nc.tensor.matmul(ps[:, :cw],
                                                 g_w[kk][:, m * P:(m + 1) * P],
                                                 h1a[kk][:, sl],
                                                 start=(kk == 0), stop=(kk == 3))
                            zc = zs_ch[ci_]
                            nc.vector.tensor_mul(bco[k][:, sl], zc[:, :cw],
                                                 ps[:, :cw])
                            nc.gpsimd.tensor_scalar(aco[k][:, sl], zc[:, :cw],
                                                    -1.0, 1.0, OP.mult, OP.add)
                        o_ = wk.tile([P, W], F32,
                                     tag=("hab" if reverse else "fwd"), bufs=4,
                                     name=f"scan{li}_{reverse}_{k}")
                        if reverse:
                            nc.vector.tensor_tensor_scan(
                                o_[:, ::-1], aco[k][:, ::-1], bco[k][:, ::-1],
                                0.0, OP.mult, OP.add)
                        else:
                            nc.vector.tensor_tensor_scan(
                                o_[:], aco[k][:], bco[k][:], 0.0,
                                OP.mult, OP.add)
                        outs.append(o_)
                    return outs

                fwd = gru_pass(gf_w, False, i)
                bwd = gru_pass(gb_w, True, i)

                prod = [wk.tile([P, W], F32R, tag="bco", bufs=4,
                                name=f"prod{i}_{k}") for k in range(4)]
                for (lo, cw) in CHUNKS:
                    sl = slice(lo, lo + cw)
                    for k in range(4):
                        h1s = wk.tile([P, CH], F32, tag="zsc", bufs=5,
                                      name=f"h1s{i}_{lo}_{k}")
                        nc.vector.tensor_add(h1s[:, :cw], fwd[k][:, sl],
                                             bwd[k][:, sl])
                        nc.vector.tensor_mul(prod[k][:, sl], h1s[:, :cw],
                                             gsil[k][:, sl])
                    for m in range(2):
                        ps = pmm.tile([P, CH], F32, tag="ps",
                                      name=f"o_ps{i}_{lo}_{m}")
                        for kk in range(4):
                            nc.tensor.matmul(ps[:, :cw],
                                             out_w[kk][:, m * P:(m + 1) * P],
                                             prod[kk][:, sl],
                                             start=(kk == 0), stop=(kk == 3))
                        nc.vector.scalar_tensor_tensor(
                            h_t[m][:, sl], xn2[m][:, sl], ALPHA, ps[:, :cw],
                            OP.mult, OP.add)

            Wo_t = []
            for m in range(2):
                t = cpool.tile([P, DIM], F32R, tag=f"Wo{m}", name=f"Wo_t{m}")
                nc.sync.dma_start(t[:], WoT_in[m * P:(m + 1) * P, :])
                Wo_t.append(t)
            h_r = []
            for m in range(2):
                t = wk.tile([P, W], F32R, tag="scr", bufs=4, name=f"h_r{m}")
                nc.scalar.activation(t[:], h_t[m][:], AF.Copy)
                h_r.append(t)
            o_sb = wk.tile([DIM, 1024], F32, tag="scr", bufs=4, name="o_sb")
            for (c0, cw) in ((0, 384), (384, 384), (768, 256)):
                ps = pmm.tile([DIM, 384], F32, tag="ps", name=f"head_ps{c0}")
                for m in range(2):
                    nc.tensor.matmul(ps[:, :cw], Wo_t[m][:],
                                     h_r[m][:, PAD + c0:PAD + c0 + cw],
                                     start=(m == 0), stop=(m == 1))
                nc.scalar.activation(o_sb[:, c0:c0 + cw], ps[:, :cw], AF.Copy)
            nc.sync.dma_start(o_out[:], o_sb[:])

    nc.compile()
    return nc


def _get_prog():
    global _PROG
    if _PROG is None:
        _PROG = _build_program()
    return _PROG


def kernel(**inputs):
    from concourse.bass_utils import run_bass_kernel_spmd

    w = _prep_weights(inputs)
    in_maps = []
    for b in range(B):
        for half in (0, 1):
            in_maps.append(_core_inputs(inputs, w, b, half))
    nc = _get_prog()
    res = run_bass_kernel_spmd(nc, in_maps, list(range(8)))
    o = np.zeros((B, DIM, L), np.float32)
    u = np.zeros((B,), np.float32)
    for ci, r in enumerate(res.results):
        b, half = divmod(ci, 2)
        o[b, :, half * 1024:(half + 1) * 1024] = r["o"]
        if half == 0:
            u[b] = r["u"][0, 0]
    return o, u


# revision 10
# speedup vs baseline: 1.9300x; 1.0044x over previous
import numpy as np

B, L = 4, 2048
DIM, A_DIM, H_DIM, DEPTH = 32, 128, 256, 8
RFF, C_DIM = 64, 256
H = 512
PAD = 32
W = PAD + 1024 + PAD
CHUNKS = ((0, 384), (384, 384), (768, 320))
NCH = 3
CH = 384
P = 128
MAGIC = float(12582912.0)

_PROG = None


def _tf32(x):
    x = np.ascontiguousarray(x, np.float32)
    u = x.view(np.uint32)
    r = ((u.astype(np.uint64) + 0x1000 + ((u >> 13) & 1)) & ~np.uint64(0x1FFF))
    return r.astype(np.uint32).view(np.float32)


def _wnorm(w):
    n = np.sqrt(np.sum(w * w, axis=tuple(range(1, w.ndim)), keepdims=True))
    return (w / (n + 1e-8)).astype(np.float32)


def _prep_weights(inp):
    f32 = lambda x: np.asarray(x, np.float32)
    out = {}
    ca = np.float32(0.5 * np.sqrt(66.0 / 32.0))
    cb = np.float32(0.5 * np.sqrt(66.0 / 1.0))
    Wh = _wnorm(f32(inp['W_h']))
    out['WxT'] = _tf32((Wh[:, :DIM] * ca).T)
    out['sbias'] = np.ascontiguousarray(Wh[:, DIM] * cb)[:, None]

    out['freq'] = np.ascontiguousarray(f32(inp['rff_freq']))[:, None]
    out['phase'] = np.ascontiguousarray(f32(inp['rff_phase']) + 0.25)[:, None]
    SQ2 = np.float32(np.sqrt(2.0))
    out['WeT'] = _tf32(_wnorm(f32(inp['W_e'])).T)
    out['WlT'] = _tf32((_wnorm(f32(inp['W_label'])) / SQ2).T)
    out['wurow'] = np.ascontiguousarray((_wnorm(f32(inp['W_u'])) * SQ2))

    ca2 = np.float32(0.5 * np.sqrt(3.0))
    cb2 = np.float32(0.5 * np.sqrt(6.0))
    IS = np.float32(1.0 / 0.596)
    hgT, pcT, gfT, gbT, outT, dw = [], [], [], [], [], []
    for i in range(DEPTH):
        hg = _wnorm(f32(inp['hg_W'][i]))
        hg_eff = np.concatenate([hg[:, :H_DIM] * (ca2 * IS), hg[:, H_DIM:] * cb2], 1)
        hgT.append(_tf32(hg_eff.T))
        pc = _wnorm(f32(inp['projc_W'][i])) * (f32(inp['projc_gain'][i]) * IS)
        pcT.append(_tf32(pc.T))
        gfT.append(_tf32((_wnorm(f32(inp['gruf_W'][i])) * IS).T))
        gbT.append(_tf32((_wnorm(f32(inp['grub_W'][i])) * IS).T))
        sc = np.float32(0.3 / (np.sqrt(0.58) * np.sqrt(2.0) * 0.596))
        outT.append(_tf32((_wnorm(f32(inp['out_W'][i])) * sc).T))
        dw.append(_wnorm(f32(inp['dw_W'][i]))[:, 0, :])
    out['hgT'] = np.ascontiguousarray(np.stack(hgT))
    out['pcT'] = np.ascontiguousarray(np.stack(pcT))
    out['gfT'] = np.ascontiguousarray(np.stack(gfT))
    out['gbT'] = np.ascontiguousarray(np.stack(gbT))
    out['outT'] = np.ascontiguousarray(np.stack(outT))
    out['dw'] = np.ascontiguousarray(np.stack(dw, axis=1).reshape(H, DEPTH * 3))
    out['WoT'] = _tf32((_wnorm(f32(inp['Wo'])) * f32(inp['out_gain'])).T)
    return out


def _core_inputs(inp, w, b, half):
    g0 = half * 1024 - PAD
    x_pad = np.zeros((DIM, W), np.float32)
    a_pad = np.zeros((A_DIM, W), np.float32)
    mask = np.zeros((1, W), np.float32)
    lo, hi = max(g0, 0), min(g0 + W, L)
    x_pad[:, lo - g0:hi - g0] = np.asarray(inp['x'][b], np.float32)[:, lo:hi]
    a_pad[:, lo - g0:hi - g0] = np.asarray(inp['a'][b], np.float32)[:, lo:hi]
    mask[:, lo - g0:hi - g0] = 1.0
    return {
        'x': _tf32(x_pad),
        'a': _tf32(a_pad),
        'mask': np.ascontiguousarray(np.broadcast_to(mask, (P, W))),
        'tcol': np.full((RFF, 1), np.float32(inp['t'][b]), np.float32),
        'label': _tf32(np.tile(np.asarray(inp['label'][b], np.float32)[:, None], (1, 2))),
        'freq': w['freq'], 'phase': w['phase'],
        'WxT': w['WxT'], 'sbias': w['sbias'],
        'WeT': w['WeT'], 'WlT': w['WlT'], 'WoT': w['WoT'],
        'frow': w['freq'].T.copy(), 'prow': w['phase'].T.copy(),
        'wurow': w['wurow'],
        'dw': w['dw'], 'hgT': w['hgT'], 'pcT': w['pcT'],
        'gfT': w['gfT'], 'gbT': w['gbT'], 'outT': w['outT'],
        'ones': np.ones((P, P), np.float32),
    }


def _build_program():
    import concourse.mybir as mybir
    import concourse.tile as tile
    from concourse import bacc

    F32 = mybir.dt.float32
    F32R = mybir.dt.float32r
    AF = mybir.ActivationFunctionType
    OP = mybir.AluOpType

    nc = bacc.Bacc(None)

    dp = nc.declare_dram_parameter
    x_in = dp("x", [DIM, W], F32R, isOutput=False)
    a_in = dp("a", [A_DIM, W], F32R, isOutput=False)
    mask_in = dp("mask", [P, W], F32, isOutput=False)
    tcol_in = dp("tcol", [RFF, 1], F32, isOutput=False)
    label_in = dp("label", [5, 2], F32R, isOutput=False)
    freq_in = dp("freq", [RFF, 1], F32, isOutput=False)
    phase_in = dp("phase", [RFF, 1], F32, isOutput=False)
    WxT_in = dp("WxT", [DIM, H_DIM], F32R, isOutput=False)
    sbias_in = dp("sbias", [H_DIM, 1], F32, isOutput=False)
    WeT_in = dp("WeT", [RFF, C_DIM], F32R, isOutput=False)
    WlT_in = dp("WlT", [5, C_DIM], F32R, isOutput=False)
    frow_in = dp("frow", [1, RFF], F32, isOutput=False)
    prow_in = dp("prow", [1, RFF], F32, isOutput=False)
    wurow_in = dp("wurow", [1, RFF], F32, isOutput=False)
    WoT_in = dp("WoT", [H_DIM, DIM], F32R, isOutput=False)
    dw_in = dp("dw", [H, DEPTH * 3], F32, isOutput=False)
    hgT_in = dp("hgT", [DEPTH, 384, 1024], F32R, isOutput=False)
    pcT_in = dp("pcT", [DEPTH, C_DIM, 1024], F32R, isOutput=False)
    gfT_in = dp("gfT", [DEPTH, H, 1024], F32R, isOutput=False)
    gbT_in = dp("gbT", [DEPTH, H, 1024], F32R, isOutput=False)
    outT_in = dp("outT", [DEPTH, H, H_DIM], F32R, isOutput=False)
    ones_in = dp("ones", [P, P], F32R, isOutput=False)
    o_out = dp("o", [DIM, 1024], F32, isOutput=True)
    u_out = dp("u", [1, 1], F32, isOutput=True)

    TWO_PI = float(2.0 * np.pi)
    ALPHA = float(0.7 / np.sqrt(0.58))

    with tile.TileContext(nc) as tc:
        with (
            tc.tile_pool(name="const", bufs=1) as cpool,
            tc.tile_pool(name="state", bufs=1) as spool,
            tc.tile_pool(name="wts", bufs=1) as wpool,
            tc.tile_pool(name="work", bufs=1) as wk,
            tc.tile_pool(name="mm", bufs=6, space="PSUM") as pmm,
            tc.tile_pool(name="vec", bufs=2, space="PSUM") as pvec,
        ):
            ones = cpool.tile([P, P], F32R, tag="ones", name="ones")
            nc.sync.dma_start(ones[:], ones_in[:])
            mask = cpool.tile([P, W], F32, tag="mask", name="mask")
            nc.sync.dma_start(mask[:], mask_in[:])
            eps_col = cpool.tile([P, 1], F32, tag="eps", name="eps_col")
            nc.vector.memset(eps_col[:], 1e-4)
            dw_t = []
            for k in range(4):
                t = cpool.tile([P, DEPTH * 3], F32, tag=f"dw{k}", name=f"dwt{k}")
                nc.sync.dma_start(t[:], dw_in[k * P:(k + 1) * P, :])
                dw_t.append(t)
            a_t = cpool.tile([P, W], F32R, tag="a", name="a_t")
            nc.sync.dma_start(a_t[:], a_in[:])
            sbias = []
            for m in range(2):
                t = cpool.tile([P, 1], F32, tag=f"sb{m}", name=f"sbias{m}")
                nc.sync.dma_start(t[:], sbias_in[m * P:(m + 1) * P, :])
                sbias.append(t)

            tcol = cpool.tile([RFF, 1], F32, tag="tcol", name="tcol")
            nc.sync.dma_start(tcol[:], tcol_in[:])
            freq = cpool.tile([RFF, 1], F32, tag="freq", name="freq")
            nc.sync.dma_start(freq[:], freq_in[:])
            phase = cpool.tile([RFF, 1], F32, tag="phase", name="phase")
            nc.sync.dma_start(phase[:], phase_in[:])
            label2 = cpool.tile([5, 2], F32R, tag="label", name="label2")
            nc.sync.dma_start(label2[:], label_in[:])
            WeT = wk.tile([RFF, C_DIM], F32R, tag="scr", bufs=4, name="WeT")
            nc.sync.dma_start(WeT[:], WeT_in[:])
            WlT = wk.tile([5, C_DIM], F32R, tag="scr", bufs=4, name="WlT")
            nc.sync.dma_start(WlT[:], WlT_in[:])
            frow = cpool.tile([1, RFF], F32, tag="frow", name="frow")
            nc.sync.dma_start(frow[:], frow_in[:])
            prow = cpool.tile([1, RFF], F32, tag="prow", name="prow")
            nc.sync.dma_start(prow[:], prow_in[:])
            wurow = cpool.tile([1, RFF], F32, tag="wurow", name="wurow")
            nc.sync.dma_start(wurow[:], wurow_in[:])

            xarg = wk.tile([RFF, 1], F32, tag="zsc", bufs=5, name="xarg")
            nc.vector.scalar_tensor_tensor(xarg[:], freq[:], tcol[:], phase[:],
                                           OP.mult, OP.add)
            m1 = wk.tile([RFF, 1], F32, tag="zsc", bufs=5, name="m1")
            nc.vector.tensor_scalar_add(m1[:], xarg[:], MAGIC)
            m2 = wk.tile([RFF, 1], F32, tag="zsc", bufs=5, name="m2")
            nc.vector.tensor_scalar_sub(m2[:], m1[:], MAGIC)
            red = wk.tile([RFF, 1], F32, tag="zsc", bufs=5, name="red")
            nc.vector.tensor_sub(red[:], xarg[:], m2[:])
            fsin = wk.tile([RFF, 2], F32R, tag="zsc", bufs=5, name="fsin")
            nc.scalar.activation(fsin[:, 0:1], red[:], AF.Sin, scale=TWO_PI)
            nc.scalar.activation(fsin[:, 1:2], red[:], AF.Sin, scale=TWO_PI)

            xargr = wk.tile([1, RFF], F32, tag="zsc", bufs=5, name="xargr")
            nc.vector.scalar_tensor_tensor(xargr[:], frow[:], tcol[:1, :], prow[:],
                                           OP.mult, OP.add)
            m1r = wk.tile([1, RFF], F32, tag="zsc", bufs=5, name="m1r")
            nc.vector.tensor_scalar_add(m1r[:], xargr[:], MAGIC)
            m2r = wk.tile([1, RFF], F32, tag="zsc", bufs=5, name="m2r")
            nc.vector.tensor_scalar_sub(m2r[:], m1r[:], MAGIC)
            redr = wk.tile([1, RFF], F32, tag="zsc", bufs=5, name="redr")
            nc.vector.tensor_sub(redr[:], xargr[:], m2r[:])
            sinr = wk.tile([1, RFF], F32, tag="zsc", bufs=5, name="sinr")
            nc.scalar.activation(sinr[:], redr[:], AF.Sin, scale=TWO_PI)
            usum = wk.tile([1, RFF], F32, tag="zsc", bufs=5, name="usum")
            nc.vector.tensor_mul(usum[:], wurow[:], sinr[:])
            usb = wk.tile([1, 1], F32, tag="zsc", bufs=5, name="usb")
            nc.vector.tensor_reduce(usb[:], usum[:], mybir.AxisListType.X,
                                    OP.add)
            nc.sync.dma_start(u_out[:], usb[:])

            c_t = []
            for m in range(2):
                cps = pvec.tile([P, 2], F32, tag="vps", name=f"cps{m}")
                nc.tensor.matmul(cps[:], WeT[:, m * P:(m + 1) * P], fsin[:],
                                 start=True, stop=False)
                nc.tensor.matmul(cps[:], WlT[:, m * P:(m + 1) * P], label2[:],
                                 start=False, stop=True)
                ct = cpool.tile([P, 2], F32R, tag=f"c{m}", name=f"c_t{m}")
                nc.scalar.activation(ct[:], cps[:], AF.Silu)
                c_t.append(ct)

            cm_all = cpool.tile([P, DEPTH * 8], F32, tag="cm", name="cm_all")
            for i in range(DEPTH):
                pc0 = wpool.tile([P, 1024], F32R, tag="wgf", bufs=4,
                                 name=f"pc0_{i}")
                nc.sync.dma_start(pc0[:], pcT_in[i, 0:P, :])
                pc1 = wpool.tile([P, 1024], F32R, tag="wgf", bufs=4,
                                 name=f"pc1_{i}")
                nc.sync.dma_start(pc1[:], pcT_in[i, P:C_DIM, :])
                for m in range(8):
                    cps = pvec.tile([P, 2], F32, tag="vps", name=f"cmps{i}_{m}")
                    nc.tensor.matmul(cps[:], pc0[:, m * P:(m + 1) * P], c_t[0][:],
                                     start=True, stop=False)
                    nc.tensor.matmul(cps[:], pc1[:, m * P:(m + 1) * P], c_t[1][:],
                                     start=False, stop=True)
                    nc.scalar.activation(cm_all[:, i * 8 + m:i * 8 + m + 1],
                                         cps[:, 0:1], AF.Copy, bias=1.0)

            xt = wk.tile([DIM, W], F32R, tag="scr", bufs=4, name="xt")
            nc.sync.dma_start(xt[:], x_in[:])
            WxT = wk.tile([DIM, H_DIM], F32R, tag="scr", bufs=4, name="WxT")
            nc.sync.dma_start(WxT[:], WxT_in[:])
            h_t = [spool.tile([P, W], F32, tag=f"h{m}", name=f"h_t{m}")
                   for m in range(2)]
            for (lo, cw) in CHUNKS:
                sl = slice(lo, lo + cw)
                for m in range(2):
                    ps = pmm.tile([P, CH], F32, tag="ps", name=f"st_ps{lo}_{m}")
                    nc.tensor.matmul(ps[:, :cw], WxT[:, m * P:(m + 1) * P],
                                     xt[:, sl], start=True, stop=True)
                    nc.vector.scalar_tensor_tensor(h_t[m][:, sl], mask[:, sl],
                                                   sbias[m][:], ps[:, :cw],
                                                   OP.mult, OP.add)

            for i in range(DEPTH):
                hg_w = []
                for kk in range(3):
                    t = wpool.tile([P, 1024], F32R, tag="whg", bufs=3,
                                   name=f"hgw{i}_{kk}")
                    nc.sync.dma_start(t[:], hgT_in[i, kk * P:(kk + 1) * P, :])
                    hg_w.append(t)
                gf_w = []
                gb_w = []
                for kk in range(4):
                    t = wpool.tile([P, 1024], F32R, tag="wgf", bufs=4,
                                   name=f"gfw{i}_{kk}")
                    nc.sync.dma_start(t[:], gfT_in[i, kk * P:(kk + 1) * P, :])
                    gf_w.append(t)
                for kk in range(4):
                    t = wpool.tile([P, 1024], F32R, tag="wgb", bufs=4,
                                   name=f"gbw{i}_{kk}")
                    nc.sync.dma_start(t[:], gbT_in[i, kk * P:(kk + 1) * P, :])
                    gb_w.append(t)
                out_w = []
                for kk in range(4):
                    t = wpool.tile([P, H_DIM], F32R, tag="wout", bufs=4,
                                   name=f"outw{i}_{kk}")
                    nc.sync.dma_start(t[:], outT_in[i, kk * P:(kk + 1) * P, :])
                    out_w.append(t)

                hsq = [wk.tile([P, W], F32R, tag="scr", bufs=4,
                               name=f"hsq{i}_{m}") for m in range(2)]
                rinv = wk.tile([P, W], F32, tag="rinv", name=f"rinv{i}")
                hsil = [wk.tile([P, W], F32R, tag="sca", bufs=4,
                                name=f"hsil{i}_{m}") for m in range(2)]
                xn = [wk.tile([P, W], F32, tag="scr", bufs=4, name=f"xn{i}_{m}")
                      for m in range(2)]
                for (lo, cw) in CHUNKS:
                    sl = slice(lo, lo + cw)
                    for m in range(2):
                        nc.scalar.activation(hsq[m][:, sl], h_t[m][:, sl],
                                             AF.Square)
                for (lo, cw) in CHUNKS:
                    sl = slice(lo, lo + cw)
                    ps = pmm.tile([P, CH], F32, tag="ps", name=f"pn_ps{i}_{lo}")
                    nc.tensor.matmul(ps[:, :cw], ones[:], hsq[0][:, sl],
                                     start=True, stop=False)
                    nc.tensor.matmul(ps[:, :cw], ones[:], hsq[1][:, sl],
                                     start=False, stop=True)
                    tsd = wk.tile([P, CH], F32, tag="zsc", bufs=5,
                                  name=f"tsd{i}_{lo}")
                    nc.scalar.activation(tsd[:, :cw], ps[:, :cw], AF.Sqrt,
                                         bias=eps_col[:], scale=1.0 / H_DIM)
                    nc.vector.reciprocal_approx_fast(rinv[:, sl], tsd[:, :cw])
                for (lo, cw) in CHUNKS:
                    sl = slice(lo, lo + cw)
                    for m in range(2):
                        nc.vector.tensor_mul(xn[m][:, sl], h_t[m][:, sl],
                                             rinv[:, sl])
                        nc.scalar.activation(hsil[m][:, sl], xn[m][:, sl],
                                             AF.Silu)
                xn2 = []
                for m in range(2):
                    t = wk.tile([P, W], F32, tag="scr", bufs=4,
                                name=f"xn2_{i}_{m}")
                    nc.gpsimd.tensor_mul(t[:], h_t[m][:], rinv[:])
                    xn2.append(t)

                rhs3 = [hsil[0], hsil[1], a_t]
                halo = [wk.tile([P, W + 2], F32, tag="hab", bufs=4,
                                name=f"halo{i}_{k}") for k in range(4)]
                for t in halo:
                    nc.vector.memset(t[:, 0:1], 0.0)
                    nc.vector.memset(t[:, W + 1:W + 2], 0.0)
                gsil = [wk.tile([P, W], F32, tag="gsil", bufs=4,
                                name=f"gsil{i}_{k}") for k in range(4)]
                h1a = [wk.tile([P, W], F32R, tag="h1a", bufs=4,
                               name=f"h1a{i}_{k}") for k in range(4)]
                t2s = [wk.tile([P, W], F32, tag="scr", bufs=4,
                               name=f"dwt2_{i}_{k}") for k in range(4)]

                def dw_chain(k):
                    c0 = dw_t[k][:, i * 3 + 0:i * 3 + 1]
                    c1 = dw_t[k][:, i * 3 + 1:i * 3 + 2]
                    c2 = dw_t[k][:, i * 3 + 2:i * 3 + 3]
                    for (lo, cw) in CHUNKS:
                        m1_ = wk.tile([P, CH], F32, tag="zsc", bufs=5,
                                      name=f"dwm{i}_{lo}_{k}")
                        nc.scalar.activation(m1_[:, :cw],
                                             halo[k][:, 1 + lo:1 + lo + cw],
                                             AF.Copy, scale=c1)
                        t1 = wk.tile([P, CH], F32, tag="zsc", bufs=5,
                                     name=f"dwt1_{i}_{lo}_{k}")
                        nc.vector.scalar_tensor_tensor(
                            t1[:, :cw], halo[k][:, lo:lo + cw], c0, m1_[:, :cw],
                            OP.mult, OP.add)
                        nc.vector.scalar_tensor_tensor(
                            t2s[k][:, lo:lo + cw],
                            halo[k][:, 2 + lo:2 + lo + cw], c2,
                            t1[:, :cw], OP.mult, OP.add)
                    nc.vector.tensor_mul(t2s[k][:, 0:PAD + 1],
                                         t2s[k][:, 0:PAD + 1],
                                         mask[:, 0:PAD + 1])
                    nc.vector.tensor_mul(t2s[k][:, W - PAD - 1:W],
                                         t2s[k][:, W - PAD - 1:W],
                                         mask[:, W - PAD - 1:W])
                    nc.scalar.activation(h1a[k][:], t2s[k][:], AF.Silu,
                                         scale=cm_all[:, i * 8 + k:i * 8 + k + 1])

                for m in range(8):
                    for (lo, cw) in CHUNKS:
                        sl = slice(lo, lo + cw)
                        ps = pmm.tile([P, CH], F32, tag="ps",
                                      name=f"hg_ps{i}_{lo}_{m}")
                        for kk in range(3):
                            nc.tensor.matmul(ps[:, :cw],
                                             hg_w[kk][:, m * P:(m + 1) * P],
                                             rhs3[kk][:, sl],
                                             start=(kk == 0), stop=(kk == 2))
                        if m < 4:
                            nc.scalar.activation(
                                halo[m][:, 1 + lo:1 + lo + cw], ps[:, :cw],
                                AF.Copy)
                        else:
                            nc.scalar.activation(
                                gsil[m - 4][:, sl], ps[:, :cw], AF.Silu,
                                scale=cm_all[:, i * 8 + m:i * 8 + m + 1])
                    if m < 4:
                        dw_chain(m)

                def gru_pass(g_w, reverse, li):
                    bco = [wk.tile([P, W], F32, tag="bco", bufs=4,
                                   name=f"bco{li}_{reverse}_{k}")
                           for k in range(4)]
                    aco = [wk.tile([P, W], F32, tag="sca", bufs=4,
                                   name=f"aco{li}_{reverse}_{k}")
                           for k in range(4)]
                    outs = []
                    for k in range(4):
                        zs_ch = []
                        for (lo, cw) in CHUNKS:
                            ps = pmm.tile([P, CH], F32, tag="ps",
                                          name=f"gz_ps{li}_{reverse}_{lo}_{k}")
                            for kk in range(4):
                                nc.tensor.matmul(ps[:, :cw],
                                                 g_w[kk][:, k * P:(k + 1) * P],
                                                 h1a[kk][:, lo:lo + cw],
                                                 start=(kk == 0), stop=(kk == 3))
                            zc = wk.tile([P, CH], F32, tag="zsc", bufs=5,
                                         name=f"zc{li}_{reverse}_{lo}_{k}")
                            nc.scalar.activation(zc[:, :cw], ps[:, :cw],
                                                 AF.Sigmoid)
                            zs_ch.append(zc)
                        for ci_, (lo, cw) in enumerate(CHUNKS):
                            sl = slice(lo, lo + cw)
                            m = k + 4
                            ps = pmm.tile([P, CH], F32, tag="ps",
                                          name=f"gh_ps{li}_{reverse}_{lo}_{k}")
                            for kk in range(4):
                                nc.tensor.matmul(ps[:, :cw],
                                                 g_w[kk][:, m * P:(m + 1) * P],
                                                 h1a[kk][:, sl],
                                                 start=(kk == 0), stop=(kk == 3))
                            zc = zs_ch[ci_]
                            nc.vector.tensor_mul(bco[k][:, sl], zc[:, :cw],
                                                 ps[:, :cw])
                            nc.gpsimd.tensor_scalar(aco[k][:, sl], zc[:, :cw],
                                                    -1.0, 1.0, OP.mult, OP.add)
                        o_ = wk.tile([P, W], F32,
                                     tag=("hab" if reverse else "fwd"), bufs=4,
                                     name=f"scan{li}_{reverse}_{k}")
                        if reverse:
                            nc.vector.tensor_tensor_scan(
                                o_[:, ::-1], aco[k][:, ::-1], bco[k][:, ::-1],
                                0.0, OP.mult, OP.add)
                        else:
                            nc.vector.tensor_tensor_scan(
                                o_[:], aco[k][:], bco[k][:], 0.0,
                                OP.mult, OP.add)
                        outs.append(o_)
                    return outs

                fwd = gru_pass(gf_w, False, i)
                bwd = gru_pass(gb_w, True, i)

                prod = [wk.tile([P, W], F32R, tag="bco", bufs=4,
                                name=f"prod{i}_{k}") for k in range(4)]
                for (lo, cw) in CHUNKS:
                    sl = slice(lo, lo + cw)
                    for k in range(4):
                        h1s = wk.tile([P, CH], F32, tag="zsc", bufs=5,
                                      name=f"h1s{i}_{lo}_{k}")
                        nc.gpsimd.tensor_add(h1s[:, :cw], fwd[k][:, sl],
                                               bwd[k][:, sl])
                        nc.vector.tensor_mul(prod[k][:, sl], h1s[:, :cw],
                                             gsil[k][:, sl])
                    for m in range(2):
                        ps = pmm.tile([P, CH], F32, tag="ps",
                                      name=f"o_ps{i}_{lo}_{m}")
                        for kk in range(4):
                            nc.tensor.matmul(ps[:, :cw],
                                             out_w[kk][:, m * P:(m + 1) * P],
                                             prod[kk][:, sl],
                                             start=(kk == 0), stop=(kk == 3))
                        nc.vector.scalar_tensor_tensor(
                            h_t[m][:, sl], xn2[m][:, sl], ALPHA, ps[:, :cw],
                            OP.mult, OP.add)

            Wo_t = []
            for m in range(2):
                t = cpool.tile([P, DIM], F32R, tag=f"Wo{m}", name=f"Wo_t{m}")
                nc.sync.dma_start(t[:], WoT_in[m * P:(m + 1) * P, :])
                Wo_t.append(t)
            h_r = []
            for m in range(2):
                t = wk.tile([P, W], F32R, tag="scr", bufs=4, name=f"h_r{m}")
                nc.scalar.activation(t[:], h_t[m][:], AF.Copy)
                h_r.append(t)
            o_sb = wk.tile([DIM, 1024], F32, tag="scr", bufs=4, name="o_sb")
            for (c0, cw) in ((0, 384), (384, 384), (768, 256)):
                ps = pmm.tile([DIM, 384], F32, tag="ps", name=f"head_ps{c0}")
                for m in range(2):
                    nc.tensor.matmul(ps[:, :cw], Wo_t[m][:],
                                     h_r[m][:, PAD + c0:PAD + c0 + cw],
                                     start=(m == 0), stop=(m == 1))
                nc.scalar.activation(o_sb[:, c0:c0 + cw], ps[:, :cw], AF.Copy)
            nc.sync.dma_start(o_out[:], o_sb[:])

    nc.compile()
    return nc


def _get_prog():
    global _PROG
    if _PROG is None:
        _PROG = _build_program()
    return _PROG


def kernel(**inputs):
    from concourse.bass_utils import run_bass_kernel_spmd

    w = _prep_weights(inputs)
    in_maps = []
    for b in range(B):
        for half in (0, 1):
            in_maps.append(_core_inputs(inputs, w, b, half))
    nc = _get_prog()
    res = run_bass_kernel_spmd(nc, in_maps, list(range(8)))
    o = np.zeros((B, DIM, L), np.float32)
    u = np.zeros((B,), np.float32)
    for ci, r in enumerate(res.results):
        b, half = divmod(ci, 2)
        o[b, :, half * 1024:(half + 1) * 1024] = r["o"]
        if half == 0:
            u[b] = r["u"][0, 0]
    return o, u
